# revision 7
# baseline (speedup 1.0000x reference)
import numpy as np
import concourse.bass as bass
import concourse.tile as tile
from concourse import mybir
from concourse.bass_utils import run_bass_kernel_spmd
from concourse.masks import make_identity

P = 128
S = 2048
D = 512
U = 1024
NS = S // P      # 16 s-tiles
ND = D // P      # 4 d-blocks
NU = U // P      # 8 u-blocks
NEG = -60000.0
EPS = 1e-6


def _patched_drain_and_barrier(self, tick_clock, wait_clock):
    nc = self.nc
    probe = nc.sync.nop(nofuse=True, hint="drain_waits_probe")
    wait_clock.add_sem_waits(probe.ins, tile.ScopedClock({None: tick_clock.global_clock}))
    si = probe.ins.sync_info
    waits = list(si.on_wait) if si is not None else []
    assert self.sems is not None
    handles = {h.name: h for h in self.sems.allocated().values()}
    if len(waits) > 1:
        import bass_rust
        probe.ins.sync_info = bass_rust.SyncInfo(on_wait=waits[:1], on_update=[])
        for w in waits[1:]:
            h = handles.get(w.ant_name)
            assert h is not None, (w.ant_name, list(handles))
            nc.sync.wait_ge(h, w.wait_value)
    nc.sync.drain()
    nc.all_engine_barrier()
    popped = nc._tile_sem_poison_stack.pop()
    assert popped is self._sem_poison
    nc.clear_and_free_semaphores(list(self.sems.allocated().values()))
    nc.all_engine_barrier()


tile.TileContext._drain_and_barrier = _patched_drain_and_barrier

# The walrus backend in this toolchain rejects instructions carrying more
# than one semaphore wait ("Too many sync wait commands"). Split excess
# waits onto single-wait NoOp carriers on the same engine, which execute
# in order ahead of the real instruction.
_MAXW = 1
_orig_lower_ordered = tile.TileContext._lower_ordered_insts


def _patched_lower_ordered(self, ordered):
    nc = self.nc
    for insts in ordered.values():
        out = []
        for inst in insts:
            si = getattr(inst, "sync_info", None)
            eng = getattr(inst, "engine", None)
            if (si is not None and si.on_wait and len(si.on_wait) > _MAXW
                    and eng is not None
                    and not type(inst).__name__.startswith("BassTile")):
                waits = list(si.on_wait)
                for w in waits[:-_MAXW]:
                    out.append(mybir.InstNoOp(
                        name=nc.get_next_instruction_name(),
                        engine=eng,
                        ins=[],
                        outs=[],
                        bass_nofuse=True,
                        sync_info=mybir.SyncInfo(on_wait=[w], on_update=[]),
                    ))
                inst.sync_info = mybir.SyncInfo(
                    on_wait=waits[-_MAXW:], on_update=list(si.on_update))
            out.append(inst)
        insts[:] = out
    return _orig_lower_ordered(self, ordered)


tile.TileContext._lower_ordered_insts = _patched_lower_ordered

f32 = mybir.dt.float32
f16 = mybir.dt.float16


def _build():
    nc = bass.Bass()
    x_ext = nc.declare_dram_parameter("x", [S, D], f32, isOutput=False)
    g_ext = nc.declare_dram_parameter("gamma", [1, D], f32, isOutput=False)
    b_ext = nc.declare_dram_parameter("beta", [1, D], f32, isOutput=False)
    wq_ext = nc.declare_dram_parameter("wq", [2 * D, U], f16, isOutput=False)
    wk_ext = nc.declare_dram_parameter("wk", [2 * D, U], f16, isOutput=False)
    wv_ext = nc.declare_dram_parameter("wv", [2 * D, U], f16, isOutput=False)
    wo_ext = nc.declare_dram_parameter("wo", [2 * U, D], f16, isOutput=False)
    m_ext = nc.declare_dram_parameter("mask", [P, 4 * D], f32, isOutput=False)
    out_ext = nc.declare_dram_parameter("out", [S, D], f32, isOutput=True)

    with tile.TileContext(nc) as tc:
        with tc.tile_pool(name="const", bufs=1) as cp, \
             tc.tile_pool(name="xnt", bufs=1) as xp, \
             tc.tile_pool(name="wp", bufs=1) as wp, \
             tc.tile_pool(name="wop", bufs=1) as wop, \
             tc.tile_pool(name="qkv", bufs=1) as qp, \
             tc.tile_pool(name="ln", bufs=2) as lp, \
             tc.tile_pool(name="att", bufs=2) as ap_, \
             tc.tile_pool(name="st", bufs=2) as sp, \
             tc.tile_pool(name="oacc", bufs=1) as op, \
             tc.tile_pool(name="outp", bufs=2) as up, \
             tc.tile_pool(name="mm", bufs=2, space="PSUM") as mmp, \
             tc.tile_pool(name="sc", bufs=2, space="PSUM") as scp, \
             tc.tile_pool(name="pv", bufs=1, space="PSUM") as pvp, \
             tc.tile_pool(name="tr", bufs=2, space="PSUM") as trp:

            ident = cp.tile([P, P], f16, tag="ident")
            make_identity(nc, ident[:])
            gam = cp.tile([P, D], f32, tag="gam")
            bet = cp.tile([P, D], f32, tag="bet")
            g_ap = g_ext[:, :]
            nc.gpsimd.dma_start(out=gam[:], in_=bass.AP(
                tensor=g_ap.tensor, offset=g_ap.offset, ap=[[0, P], g_ap.ap[-1]]))
            b_ap = b_ext[:, :]
            nc.gpsimd.dma_start(out=bet[:], in_=bass.AP(
                tensor=b_ap.tensor, offset=b_ap.offset, ap=[[0, P], b_ap.ap[-1]]))
            eps = cp.tile([P, 1], f32, tag="eps")
            nc.vector.memset(eps[:], EPS)
            mask = cp.tile([P, 4 * D], f32, tag="mask")
            nc.sync.dma_start(out=mask[:], in_=m_ext[:, :])

            xnT = [xp.tile([P, S], f16, tag=f"xnt{j}", name=f"xnt{j}") for j in range(ND)]
            oacc = [op.tile([P, D], f16, tag=f"oacc{i}", name=f"oacc{i}") for i in range(NS)]

            # ---- LayerNorm + transpose ----
            for i in range(NS):
                xt = lp.tile([P, D], f32, tag="x")
                nc.sync.dma_start(out=xt[:], in_=x_ext[i * P:(i + 1) * P, :])
                stats = lp.tile([P, 6], f32, tag="bs")
                nc.vector.bn_stats(out=stats[:], in_=xt[:])
                mv = lp.tile([P, 2], f32, tag="mv")
                nc.vector.bn_aggr(out=mv[:], in_=stats[:])
                sd = lp.tile([P, 1], f32, tag="sd")
                nc.scalar.activation(out=sd[:], in_=mv[:, 1:2],
                                     func=mybir.ActivationFunctionType.Sqrt,
                                     bias=eps[:], scale=1.0, alpha=0.0)
                nc.vector.reciprocal(out=sd[:], in_=sd[:])
                nc.vector.tensor_scalar(out=xt[:], in0=xt[:],
                                        scalar1=mv[:, 0:1], scalar2=sd[:],
                                        op0=mybir.AluOpType.subtract,
                                        op1=mybir.AluOpType.mult)
                nc.vector.tensor_mul(out=xt[:], in0=xt[:], in1=gam[:])
                xh = lp.tile([P, D], f16, tag="xh")
                nc.vector.tensor_add(out=xh[:], in0=xt[:], in1=bet[:])
                for j in range(ND):
                    tp = trp.tile([P, P], f16, tag="tr")
                    nc.tensor.transpose(tp[:], xh[:, j * P:(j + 1) * P], ident[:])
                    nc.any.tensor_copy(out=xnT[j][:, i * P:(i + 1) * P], in_=tp[:])

            for h in range(2):
                # ---- Q^T and K^T ----
                QT = [qp.tile([P, S], f16, tag=f"qt{u}", name=f"qt{u}") for u in range(NU)]
                KT = [qp.tile([P, S], f16, tag=f"kt{u}", name=f"kt{u}") for u in range(NU)]
                V = [qp.tile([P, U], f16, tag=f"v{t}", name=f"v{t}") for t in range(NS)]

                for name, w_ext_, dst in (("q", wq_ext, QT), ("k", wk_ext, KT)):
                    wt = [wp.tile([P, U], f16, tag=f"w{j}", name=f"w{j}") for j in range(ND)]
                    for j in range(ND):
                        nc.sync.dma_start(
                            out=wt[j][:],
                            in_=w_ext_[h * D + j * P: h * D + (j + 1) * P, :])
                    for u in range(NU):
                        for sl in range(S // 512):
                            mm = mmp.tile([P, 512], f32, tag="mm")
                            for j in range(ND):
                                nc.tensor.matmul(mm[:],
                                                 wt[j][:, u * P:(u + 1) * P],
                                                 xnT[j][:, sl * 512:(sl + 1) * 512],
                                                 start=(j == 0), stop=(j == ND - 1))
                            nc.any.tensor_copy(out=dst[u][:, sl * 512:(sl + 1) * 512], in_=mm[:])

                # ---- V ----
                wt = [wp.tile([P, U], f16, tag=f"w{j}", name=f"w{j}") for j in range(ND)]
                for j in range(ND):
                    nc.sync.dma_start(
                        out=wt[j][:],
                        in_=wv_ext[h * D + j * P: h * D + (j + 1) * P, :])
                for t in range(NS):
                    for us in range(2):
                        mm = mmp.tile([P, 512], f32, tag="mm")
                        for j in range(ND):
                            nc.tensor.matmul(mm[:],
                                             xnT[j][:, t * P:(t + 1) * P],
                                             wt[j][:, us * 512:(us + 1) * 512],
                                             start=(j == 0), stop=(j == ND - 1))
                        nc.any.tensor_copy(out=V[t][:, us * 512:(us + 1) * 512], in_=mm[:])

                # ---- Wout tiles ----
                wo_t = [wop.tile([P, D], f16, tag=f"wo{ub}", name=f"wo{ub}") for ub in range(NU)]
                for ub in range(NU):
                    nc.sync.dma_start(
                        out=wo_t[ub][:],
                        in_=wo_ext[h * U + ub * P: h * U + (ub + 1) * P, :])

                # ---- attention per s-tile ----
                for i in range(NS):
                    nch = i // 4 + 1
                    Pt = ap_.tile([P, S], f16, tag="P")
                    mneg = sp.tile([P, 4], f32, tag="mneg")
                    rsum = sp.tile([P, 4], f32, tag="rsum")
                    for c in range(nch):
                        sc = scp.tile([P, 512], f32, tag="sc")
                        for u in range(NU):
                            nc.tensor.matmul(sc[:],
                                             QT[u][:, i * P:(i + 1) * P],
                                             KT[u][:, c * 512:(c + 1) * 512],
                                             start=(u == 0), stop=(u == NU - 1))
                        if c == i // 4:
                            m = i % 4
                            nc.vector.tensor_add(out=sc[:], in0=sc[:],
                                                 in1=mask[:, m * 512:(m + 1) * 512])
                        nc.vector.reduce_max(out=mneg[:, c:c + 1], in_=sc[:],
                                             axis=mybir.AxisListType.X, negate=True)
                        nc.scalar.activation(out=Pt[:, c * 512:(c + 1) * 512], in_=sc[:],
                                             func=mybir.ActivationFunctionType.Exp,
                                             bias=mneg[:, c:c + 1], scale=1.0,
                                             accum_out=rsum[:, c:c + 1])
                    # global softmax rescale: beta_c = exp(m_c - m_g) / Z
                    mpos = sp.tile([P, 4], f32, tag="mpos")
                    nc.vector.tensor_scalar_mul(out=mpos[:, 0:nch], in0=mneg[:, 0:nch],
                                                scalar1=-1.0)
                    mgn = sp.tile([P, 1], f32, tag="mgn")
                    nc.vector.reduce_max(out=mgn[:], in_=mpos[:, 0:nch],
                                         axis=mybir.AxisListType.X, negate=True)
                    alph = sp.tile([P, 4], f32, tag="alph")
                    nc.scalar.activation(out=alph[:, 0:nch], in_=mneg[:, 0:nch],
                                         func=mybir.ActivationFunctionType.Exp,
                                         bias=mgn[:], scale=-1.0)
                    pr = sp.tile([P, 4], f32, tag="pr")
                    nc.vector.tensor_mul(out=pr[:, 0:nch], in0=rsum[:, 0:nch],
                                         in1=alph[:, 0:nch])
                    tot = sp.tile([P, 1], f32, tag="tot")
                    nc.vector.reduce_sum(out=tot[:], in_=pr[:, 0:nch],
                                         axis=mybir.AxisListType.X)
                    nc.vector.reciprocal(out=tot[:], in_=tot[:])
                    bt = sp.tile([P, 4], f32, tag="bt")
                    nc.vector.tensor_scalar_mul(out=bt[:, 0:nch], in0=alph[:, 0:nch],
                                                scalar1=tot[:])
                    for c in range(nch):
                        nc.vector.tensor_scalar_mul(out=Pt[:, c * 512:(c + 1) * 512],
                                                    in0=Pt[:, c * 512:(c + 1) * 512],
                                                    scalar1=bt[:, c:c + 1])
                    # transpose probs blocks 0..i
                    pt = ap_.tile([P, S], f16, tag="pt")
                    for tb in range(i + 1):
                        tp = trp.tile([P, P], f16, tag="tr")
                        nc.tensor.transpose(tp[:], Pt[:, tb * P:(tb + 1) * P], ident[:])
                        nc.any.tensor_copy(out=pt[:, tb * P:(tb + 1) * P], in_=tp[:])
                    # probs @ V
                    ht = ap_.tile([P, U], f16, tag="ht")
                    for us in range(2):
                        pv = pvp.tile([P, 512], f32, tag=f"pv{us}")
                        for tb in range(i + 1):
                            nc.tensor.matmul(pv[:],
                                             pt[:, tb * P:(tb + 1) * P],
                                             V[tb][:, us * 512:(us + 1) * 512],
                                             start=(tb == 0), stop=(tb == i))
                        nc.any.tensor_copy(out=ht[:, us * 512:(us + 1) * 512], in_=pv[:])
                    # transpose head-out blocks
                    htt = ap_.tile([P, U], f16, tag="htt")
                    for ub in range(NU):
                        tp = trp.tile([P, P], f16, tag="tr")
                        nc.tensor.transpose(tp[:], ht[:, ub * P:(ub + 1) * P], ident[:])
                        nc.any.tensor_copy(out=htt[:, ub * P:(ub + 1) * P], in_=tp[:])
                    # output projection
                    om = mmp.tile([P, 512], f32, tag="mm")
                    for ub in range(NU):
                        nc.tensor.matmul(om[:],
                                         htt[:, ub * P:(ub + 1) * P],
                                         wo_t[ub][:],
                                         start=(ub == 0), stop=(ub == NU - 1))
                    if h == 0:
                        nc.any.tensor_copy(out=oacc[i][:], in_=om[:])
                    else:
                        oh = up.tile([P, D], f16, tag="oh")
                        nc.any.tensor_copy(out=oh[:], in_=om[:])
                        of = up.tile([P, D], f32, tag="of")
                        nc.vector.tensor_add(out=of[:], in0=oh[:], in1=oacc[i][:])
                        nc.sync.dma_start(out=out_ext[i * P:(i + 1) * P, :], in_=of[:])
    return nc


_NC = None


def _get_nc():
    global _NC
    if _NC is None:
        _NC = _build()
    return _NC


def _mask_np():
    r = np.arange(P)[:, None]
    j = np.arange(D)[None, :]
    m = np.empty((P, 4 * D), np.float32)
    for k in range(4):
        m[:, k * D:(k + 1) * D] = np.where(j <= k * P + r, 0.0, NEG)
    return m


def _run(inputs, trace=False):
    x = np.asarray(inputs["x"], dtype=np.float32)          # [4, 2048, 512]
    gamma = np.asarray(inputs["gamma"], dtype=np.float32).reshape(1, D)
    beta = np.asarray(inputs["beta"], dtype=np.float32).reshape(1, D)
    Wq = np.asarray(inputs["Wq"], dtype=np.float32)        # [4, 512, 1024]
    Wk = np.asarray(inputs["Wk"], dtype=np.float32)
    Wv = np.asarray(inputs["Wv"], dtype=np.float32)
    Wout = np.asarray(inputs["Wout"], dtype=np.float32)    # [4096, 512]
    mask = _mask_np()

    in_maps = []
    for c in range(8):
        b, hp = c // 2, c % 2
        in_maps.append({
            "x": np.ascontiguousarray(x[b]),
            "gamma": gamma,
            "beta": beta,
            "wq": np.ascontiguousarray(Wq[2 * hp:2 * hp + 2].reshape(2 * D, U)).astype(np.float16),
            "wk": np.ascontiguousarray(Wk[2 * hp:2 * hp + 2].reshape(2 * D, U)).astype(np.float16),
            "wv": np.ascontiguousarray(Wv[2 * hp:2 * hp + 2].reshape(2 * D, U)).astype(np.float16),
            "wo": np.ascontiguousarray(Wout[2 * hp * U:(2 * hp + 2) * U]).astype(np.float16),
            "mask": mask,
        })
    res = run_bass_kernel_spmd(_get_nc(), in_maps, list(range(8)), trace=trace)
    out = np.empty((4, S, D), np.float32)
    for b in range(4):
        out[b] = res.results[2 * b]["out"] + res.results[2 * b + 1]["out"]
    return out, res


def kernel(**inputs):
    out, _ = _run(inputs, trace=False)
    return out


# revision 10
# speedup vs baseline: 1.0575x; 1.0575x over previous
import numpy as np
import concourse.bass as bass
import concourse.tile as tile
from concourse import mybir
from concourse.bass_utils import run_bass_kernel_spmd
from concourse.masks import make_identity

P = 128
S = 2048
D = 512
U = 1024
NS = S // P      # 16 s-tiles
ND = D // P      # 4 d-blocks
NU = U // P      # 8 u-blocks
NEG = -60000.0
EPS = 1e-6


def _patched_drain_and_barrier(self, tick_clock, wait_clock):
    nc = self.nc
    probe = nc.sync.nop(nofuse=True, hint="drain_waits_probe")
    wait_clock.add_sem_waits(probe.ins, tile.ScopedClock({None: tick_clock.global_clock}))
    si = probe.ins.sync_info
    waits = list(si.on_wait) if si is not None else []
    assert self.sems is not None
    handles = {h.name: h for h in self.sems.allocated().values()}
    if len(waits) > 1:
        import bass_rust
        probe.ins.sync_info = bass_rust.SyncInfo(on_wait=waits[:1], on_update=[])
        for w in waits[1:]:
            h = handles.get(w.ant_name)
            assert h is not None, (w.ant_name, list(handles))
            nc.sync.wait_ge(h, w.wait_value)
    nc.sync.drain()
    nc.all_engine_barrier()
    popped = nc._tile_sem_poison_stack.pop()
    assert popped is self._sem_poison
    nc.clear_and_free_semaphores(list(self.sems.allocated().values()))
    nc.all_engine_barrier()


tile.TileContext._drain_and_barrier = _patched_drain_and_barrier

# The walrus backend in this toolchain rejects instructions carrying more
# than one semaphore wait ("Too many sync wait commands"). Split excess
# waits onto single-wait NoOp carriers on the same engine, which execute
# in order ahead of the real instruction.
_MAXW = 1
_orig_lower_ordered = tile.TileContext._lower_ordered_insts


def _patched_lower_ordered(self, ordered):
    nc = self.nc
    for insts in ordered.values():
        out = []
        for inst in insts:
            si = getattr(inst, "sync_info", None)
            eng = getattr(inst, "engine", None)
            if (si is not None and si.on_wait and len(si.on_wait) > _MAXW
                    and eng is not None
                    and not type(inst).__name__.startswith("BassTile")):
                waits = list(si.on_wait)
                for w in waits[:-_MAXW]:
                    out.append(mybir.InstNoOp(
                        name=nc.get_next_instruction_name(),
                        engine=eng,
                        ins=[],
                        outs=[],
                        bass_nofuse=True,
                        sync_info=mybir.SyncInfo(on_wait=[w], on_update=[]),
                    ))
                inst.sync_info = mybir.SyncInfo(
                    on_wait=waits[-_MAXW:], on_update=list(si.on_update))
            out.append(inst)
        insts[:] = out
    return _orig_lower_ordered(self, ordered)


tile.TileContext._lower_ordered_insts = _patched_lower_ordered

f32 = mybir.dt.float32
f16 = mybir.dt.float16


def _build():
    nc = bass.Bass()
    x_ext = nc.declare_dram_parameter("x", [S, D], f32, isOutput=False)
    g_ext = nc.declare_dram_parameter("gamma", [1, D], f32, isOutput=False)
    b_ext = nc.declare_dram_parameter("beta", [1, D], f32, isOutput=False)
    wq_ext = nc.declare_dram_parameter("wq", [2 * D, U], f16, isOutput=False)
    wk_ext = nc.declare_dram_parameter("wk", [2 * D, U], f16, isOutput=False)
    wv_ext = nc.declare_dram_parameter("wv", [2 * D, U], f16, isOutput=False)
    wo_ext = nc.declare_dram_parameter("wo", [2 * U, D], f16, isOutput=False)
    m_ext = nc.declare_dram_parameter("mask", [P, 4 * D], f32, isOutput=False)
    out_ext = nc.declare_dram_parameter("out", [S, D], f32, isOutput=True)

    with tile.TileContext(nc) as tc:
        with tc.tile_pool(name="const", bufs=1) as cp, \
             tc.tile_pool(name="xnt", bufs=1) as xp, \
             tc.tile_pool(name="wp", bufs=1) as wp, \
             tc.tile_pool(name="wop", bufs=1) as wop, \
             tc.tile_pool(name="qkv", bufs=1) as qp, \
             tc.tile_pool(name="ln", bufs=2) as lp, \
             tc.tile_pool(name="att", bufs=2) as ap_, \
             tc.tile_pool(name="st", bufs=2) as sp, \
             tc.tile_pool(name="oacc", bufs=1) as op, \
             tc.tile_pool(name="outp", bufs=2) as up, \
             tc.tile_pool(name="mm", bufs=2, space="PSUM") as mmp, \
             tc.tile_pool(name="sc", bufs=2, space="PSUM") as scp, \
             tc.tile_pool(name="pv", bufs=1, space="PSUM") as pvp, \
             tc.tile_pool(name="tr", bufs=2, space="PSUM") as trp:

            ident = cp.tile([P, P], f16, tag="ident")
            make_identity(nc, ident[:])
            gam = cp.tile([P, D], f32, tag="gam")
            bet = cp.tile([P, D], f32, tag="bet")
            g_ap = g_ext[:, :]
            nc.gpsimd.dma_start(out=gam[:], in_=bass.AP(
                tensor=g_ap.tensor, offset=g_ap.offset, ap=[[0, P], g_ap.ap[-1]]))
            b_ap = b_ext[:, :]
            nc.gpsimd.dma_start(out=bet[:], in_=bass.AP(
                tensor=b_ap.tensor, offset=b_ap.offset, ap=[[0, P], b_ap.ap[-1]]))
            eps = cp.tile([P, 1], f32, tag="eps")
            nc.vector.memset(eps[:], EPS)
            mask = cp.tile([P, 4 * D], f32, tag="mask")
            nc.sync.dma_start(out=mask[:], in_=m_ext[:, :])

            xnT = [xp.tile([P, S], f16, tag=f"xnt{j}", name=f"xnt{j}") for j in range(ND)]
            oacc = [op.tile([P, D], f16, tag=f"oacc{i}", name=f"oacc{i}") for i in range(NS)]

            def emit_ln_tile(i):
                xt = lp.tile([P, D], f32, tag="x", name="xt")
                nc.sync.dma_start(out=xt[:], in_=x_ext[i * P:(i + 1) * P, :])
                stats = lp.tile([P, 6], f32, tag="bs", name="bs")
                nc.vector.bn_stats(out=stats[:], in_=xt[:])
                mv = lp.tile([P, 2], f32, tag="mv", name="mv")
                nc.vector.bn_aggr(out=mv[:], in_=stats[:])
                sd = lp.tile([P, 1], f32, tag="sd", name="sd")
                nc.scalar.activation(out=sd[:], in_=mv[:, 1:2],
                                     func=mybir.ActivationFunctionType.Sqrt,
                                     bias=eps[:], scale=1.0, alpha=0.0)
                nc.vector.reciprocal(out=sd[:], in_=sd[:])
                nc.vector.tensor_scalar(out=xt[:], in0=xt[:],
                                        scalar1=mv[:, 0:1], scalar2=sd[:],
                                        op0=mybir.AluOpType.subtract,
                                        op1=mybir.AluOpType.mult)
                nc.vector.tensor_mul(out=xt[:], in0=xt[:], in1=gam[:])
                xh = lp.tile([P, D], f16, tag="xh", name="xh")
                nc.vector.tensor_add(out=xh[:], in0=xt[:], in1=bet[:])
                for j in range(ND):
                    tp = trp.tile([P, P], f16, tag="tr", name="tp")
                    nc.tensor.transpose(tp[:], xh[:, j * P:(j + 1) * P], ident[:])
                    nc.any.tensor_copy(out=xnT[j][:, i * P:(i + 1) * P], in_=tp[:])

            def load_w(w_ext_, h):
                wt = [wp.tile([P, U], f16, tag=f"w{j}", name=f"w{j}") for j in range(ND)]
                for j in range(ND):
                    nc.sync.dma_start(
                        out=wt[j][:],
                        in_=w_ext_[h * D + j * P: h * D + (j + 1) * P, :])
                return wt

            def emit_proj_sl(wt, dst, sl):
                for u in range(NU):
                    mm = mmp.tile([P, 512], f32, tag="mm", name="mm")
                    for j in range(ND):
                        nc.tensor.matmul(mm[:],
                                         wt[j][:, u * P:(u + 1) * P],
                                         xnT[j][:, sl * 512:(sl + 1) * 512],
                                         start=(j == 0), stop=(j == ND - 1))
                    nc.any.tensor_copy(out=dst[u][:, sl * 512:(sl + 1) * 512], in_=mm[:])

            # ---- LayerNorm interleaved with head-0 Q projection ----
            wt0 = load_w(wq_ext, 0)
            QT0 = [qp.tile([P, S], f16, tag=f"qt{u}", name=f"qt{u}") for u in range(NU)]
            for g in range(4):
                for i in range(4 * g, 4 * g + 4):
                    emit_ln_tile(i)
                emit_proj_sl(wt0, QT0, g)

            for h in range(2):
                if h == 0:
                    QT = QT0
                else:
                    wt = load_w(wq_ext, h)
                    QT = [qp.tile([P, S], f16, tag=f"qt{u}", name=f"qt{u}") for u in range(NU)]
                    for sl in range(4):
                        emit_proj_sl(wt, QT, sl)
                KT = [qp.tile([P, S], f16, tag=f"kt{u}", name=f"kt{u}") for u in range(NU)]
                V = [qp.tile([P, U], f16, tag=f"v{t}", name=f"v{t}") for t in range(NS)]

                wt = load_w(wk_ext, h)
                for sl in range(4):
                    emit_proj_sl(wt, KT, sl)

                # ---- V ----
                wt = [wp.tile([P, U], f16, tag=f"w{j}", name=f"w{j}") for j in range(ND)]
                for j in range(ND):
                    nc.sync.dma_start(
                        out=wt[j][:],
                        in_=wv_ext[h * D + j * P: h * D + (j + 1) * P, :])
                for t in range(NS):
                    for us in range(2):
                        mm = mmp.tile([P, 512], f32, tag="mm")
                        for j in range(ND):
                            nc.tensor.matmul(mm[:],
                                             xnT[j][:, t * P:(t + 1) * P],
                                             wt[j][:, us * 512:(us + 1) * 512],
                                             start=(j == 0), stop=(j == ND - 1))
                        nc.any.tensor_copy(out=V[t][:, us * 512:(us + 1) * 512], in_=mm[:])

                # ---- Wout tiles ----
                wo_t = [wop.tile([P, D], f16, tag=f"wo{ub}", name=f"wo{ub}") for ub in range(NU)]
                for ub in range(NU):
                    nc.sync.dma_start(
                        out=wo_t[ub][:],
                        in_=wo_ext[h * U + ub * P: h * U + (ub + 1) * P, :])

                # ---- attention per s-tile ----
                for i in range(NS):
                    nch = i // 4 + 1
                    Pt = ap_.tile([P, S], f16, tag="P")
                    mneg = sp.tile([P, 4], f32, tag="mneg")
                    rsum = sp.tile([P, 4], f32, tag="rsum")
                    for c in range(nch):
                        w = (i % 4 + 1) * P if c == i // 4 else 512
                        sc = scp.tile([P, 512], f32, tag="sc")
                        for u in range(NU):
                            nc.tensor.matmul(sc[:, 0:w],
                                             QT[u][:, i * P:(i + 1) * P],
                                             KT[u][:, c * 512:c * 512 + w],
                                             start=(u == 0), stop=(u == NU - 1))
                        if c == i // 4:
                            m = i % 4
                            nc.vector.tensor_add(out=sc[:, 0:w], in0=sc[:, 0:w],
                                                 in1=mask[:, m * 512:m * 512 + w])
                        nc.vector.reduce_max(out=mneg[:, c:c + 1], in_=sc[:, 0:w],
                                             axis=mybir.AxisListType.X, negate=True)
                        nc.scalar.activation(out=Pt[:, c * 512:c * 512 + w], in_=sc[:, 0:w],
                                             func=mybir.ActivationFunctionType.Exp,
                                             bias=mneg[:, c:c + 1], scale=1.0,
                                             accum_out=rsum[:, c:c + 1])
                    # global softmax rescale: beta_c = exp(m_c - m_g) / Z
                    mpos = sp.tile([P, 4], f32, tag="mpos")
                    nc.vector.tensor_scalar_mul(out=mpos[:, 0:nch], in0=mneg[:, 0:nch],
                                                scalar1=-1.0)
                    mgn = sp.tile([P, 1], f32, tag="mgn")
                    nc.vector.reduce_max(out=mgn[:], in_=mpos[:, 0:nch],
                                         axis=mybir.AxisListType.X, negate=True)
                    alph = sp.tile([P, 4], f32, tag="alph")
                    nc.scalar.activation(out=alph[:, 0:nch], in_=mneg[:, 0:nch],
                                         func=mybir.ActivationFunctionType.Exp,
                                         bias=mgn[:], scale=-1.0)
                    pr = sp.tile([P, 4], f32, tag="pr")
                    nc.vector.tensor_mul(out=pr[:, 0:nch], in0=rsum[:, 0:nch],
                                         in1=alph[:, 0:nch])
                    tot = sp.tile([P, 1], f32, tag="tot")
                    nc.vector.reduce_sum(out=tot[:], in_=pr[:, 0:nch],
                                         axis=mybir.AxisListType.X)
                    nc.vector.reciprocal(out=tot[:], in_=tot[:])
                    bt = sp.tile([P, 4], f32, tag="bt")
                    nc.vector.tensor_scalar_mul(out=bt[:, 0:nch], in0=alph[:, 0:nch],
                                                scalar1=tot[:])
                    for c in range(nch):
                        w = (i % 4 + 1) * P if c == i // 4 else 512
                        nc.vector.tensor_scalar_mul(out=Pt[:, c * 512:c * 512 + w],
                                                    in0=Pt[:, c * 512:c * 512 + w],
                                                    scalar1=bt[:, c:c + 1])
                    # transpose probs blocks 0..i
                    pt = ap_.tile([P, S], f16, tag="pt")
                    for tb in range(i + 1):
                        tp = trp.tile([P, P], f16, tag="tr")
                        nc.tensor.transpose(tp[:], Pt[:, tb * P:(tb + 1) * P], ident[:])
                        nc.any.tensor_copy(out=pt[:, tb * P:(tb + 1) * P], in_=tp[:])
                    # probs @ V
                    ht = ap_.tile([P, U], f16, tag="ht")
                    for us in range(2):
                        pv = pvp.tile([P, 512], f32, tag=f"pv{us}")
                        for tb in range(i + 1):
                            nc.tensor.matmul(pv[:],
                                             pt[:, tb * P:(tb + 1) * P],
                                             V[tb][:, us * 512:(us + 1) * 512],
                                             start=(tb == 0), stop=(tb == i))
                        nc.any.tensor_copy(out=ht[:, us * 512:(us + 1) * 512], in_=pv[:])
                    # transpose head-out blocks
                    htt = ap_.tile([P, U], f16, tag="htt")
                    for ub in range(NU):
                        tp = trp.tile([P, P], f16, tag="tr")
                        nc.tensor.transpose(tp[:], ht[:, ub * P:(ub + 1) * P], ident[:])
                        nc.any.tensor_copy(out=htt[:, ub * P:(ub + 1) * P], in_=tp[:])
                    # output projection
                    om = mmp.tile([P, 512], f32, tag="mm")
                    for ub in range(NU):
                        nc.tensor.matmul(om[:],
                                         htt[:, ub * P:(ub + 1) * P],
                                         wo_t[ub][:],
                                         start=(ub == 0), stop=(ub == NU - 1))
                    if h == 0:
                        nc.any.tensor_copy(out=oacc[i][:], in_=om[:])
                    else:
                        oh = up.tile([P, D], f16, tag="oh")
                        nc.any.tensor_copy(out=oh[:], in_=om[:])
                        of = up.tile([P, D], f32, tag="of")
                        nc.vector.tensor_add(out=of[:], in0=oh[:], in1=oacc[i][:])
                        nc.sync.dma_start(out=out_ext[i * P:(i + 1) * P, :], in_=of[:])
    return nc


_NC = None


def _get_nc():
    global _NC
    if _NC is None:
        _NC = _build()
    return _NC


def _mask_np():
    r = np.arange(P)[:, None]
    j = np.arange(D)[None, :]
    m = np.empty((P, 4 * D), np.float32)
    for k in range(4):
        m[:, k * D:(k + 1) * D] = np.where(j <= k * P + r, 0.0, NEG)
    return m


def _run(inputs, trace=False):
    x = np.asarray(inputs["x"], dtype=np.float32)          # [4, 2048, 512]
    gamma = np.asarray(inputs["gamma"], dtype=np.float32).reshape(1, D)
    beta = np.asarray(inputs["beta"], dtype=np.float32).reshape(1, D)
    Wq = np.asarray(inputs["Wq"], dtype=np.float32)        # [4, 512, 1024]
    Wk = np.asarray(inputs["Wk"], dtype=np.float32)
    Wv = np.asarray(inputs["Wv"], dtype=np.float32)
    Wout = np.asarray(inputs["Wout"], dtype=np.float32)    # [4096, 512]
    mask = _mask_np()

    in_maps = []
    for c in range(8):
        b, hp = c // 2, c % 2
        in_maps.append({
            "x": np.ascontiguousarray(x[b]),
            "gamma": gamma,
            "beta": beta,
            "wq": np.ascontiguousarray(Wq[2 * hp:2 * hp + 2].reshape(2 * D, U)).astype(np.float16),
            "wk": np.ascontiguousarray(Wk[2 * hp:2 * hp + 2].reshape(2 * D, U)).astype(np.float16),
            "wv": np.ascontiguousarray(Wv[2 * hp:2 * hp + 2].reshape(2 * D, U)).astype(np.float16),
            "wo": np.ascontiguousarray(Wout[2 * hp * U:(2 * hp + 2) * U]).astype(np.float16),
            "mask": mask,
        })
    res = run_bass_kernel_spmd(_get_nc(), in_maps, list(range(8)), trace=trace)
    out = np.empty((4, S, D), np.float32)
    for b in range(4):
        out[b] = res.results[2 * b]["out"] + res.results[2 * b + 1]["out"]
    return out, res


def kernel(**inputs):
    out, _ = _run(inputs, trace=False)
    return out


# revision 13
# speedup vs baseline: 1.1181x; 1.0573x over previous
import numpy as np
import concourse.bass as bass
import concourse.tile as tile
from concourse import mybir
from concourse.bass_utils import run_bass_kernel_spmd
from concourse.masks import make_identity

P = 128
S = 2048
D = 512
U = 1024
NS = S // P      # 16 s-tiles
ND = D // P      # 4 d-blocks
NU = U // P      # 8 u-blocks
NEG = -60000.0
EPS = 1e-6


def _patched_drain_and_barrier(self, tick_clock, wait_clock):
    nc = self.nc
    probe = nc.sync.nop(nofuse=True, hint="drain_waits_probe")
    wait_clock.add_sem_waits(probe.ins, tile.ScopedClock({None: tick_clock.global_clock}))
    si = probe.ins.sync_info
    waits = list(si.on_wait) if si is not None else []
    assert self.sems is not None
    handles = {h.name: h for h in self.sems.allocated().values()}
    if len(waits) > 1:
        import bass_rust
        probe.ins.sync_info = bass_rust.SyncInfo(on_wait=waits[:1], on_update=[])
        for w in waits[1:]:
            h = handles.get(w.ant_name)
            assert h is not None, (w.ant_name, list(handles))
            nc.sync.wait_ge(h, w.wait_value)
    nc.sync.drain()
    nc.all_engine_barrier()
    popped = nc._tile_sem_poison_stack.pop()
    assert popped is self._sem_poison
    nc.clear_and_free_semaphores(list(self.sems.allocated().values()))
    nc.all_engine_barrier()


tile.TileContext._drain_and_barrier = _patched_drain_and_barrier

# The walrus backend in this toolchain rejects instructions carrying more
# than one semaphore wait ("Too many sync wait commands"). Split excess
# waits onto single-wait NoOp carriers on the same engine, which execute
# in order ahead of the real instruction.
_MAXW = 1
_orig_lower_ordered = tile.TileContext._lower_ordered_insts


def _patched_lower_ordered(self, ordered):
    nc = self.nc
    for insts in ordered.values():
        out = []
        for inst in insts:
            si = getattr(inst, "sync_info", None)
            eng = getattr(inst, "engine", None)
            if (si is not None and si.on_wait and len(si.on_wait) > _MAXW
                    and eng is not None
                    and not type(inst).__name__.startswith("BassTile")):
                waits = list(si.on_wait)
                for w in waits[:-_MAXW]:
                    out.append(mybir.InstNoOp(
                        name=nc.get_next_instruction_name(),
                        engine=eng,
                        ins=[],
                        outs=[],
                        bass_nofuse=True,
                        sync_info=mybir.SyncInfo(on_wait=[w], on_update=[]),
                    ))
                inst.sync_info = mybir.SyncInfo(
                    on_wait=waits[-_MAXW:], on_update=list(si.on_update))
            out.append(inst)
        insts[:] = out
    return _orig_lower_ordered(self, ordered)


tile.TileContext._lower_ordered_insts = _patched_lower_ordered

f32 = mybir.dt.float32
f16 = mybir.dt.float16


def _build():
    nc = bass.Bass()
    x_ext = nc.declare_dram_parameter("x", [S, D], f32, isOutput=False)
    g_ext = nc.declare_dram_parameter("gamma", [1, D], f32, isOutput=False)
    b_ext = nc.declare_dram_parameter("beta", [1, D], f32, isOutput=False)
    wq_ext = nc.declare_dram_parameter("wq", [2 * D, U], f16, isOutput=False)
    wk_ext = nc.declare_dram_parameter("wk", [2 * D, U], f16, isOutput=False)
    wv_ext = nc.declare_dram_parameter("wv", [2 * D, U], f16, isOutput=False)
    wo_ext = nc.declare_dram_parameter("wo", [2 * U, D], f16, isOutput=False)
    m_ext = nc.declare_dram_parameter("mask", [P, 4 * D], f32, isOutput=False)
    out_ext = nc.declare_dram_parameter("out", [S, D], f32, isOutput=True)

    with tile.TileContext(nc) as tc:
        with tc.tile_pool(name="const", bufs=1) as cp, \
             tc.tile_pool(name="xnt", bufs=1) as xp, \
             tc.tile_pool(name="wp", bufs=1) as wp, \
             tc.tile_pool(name="wop", bufs=1) as wop, \
             tc.tile_pool(name="qkv", bufs=1) as qp, \
             tc.tile_pool(name="ln", bufs=2) as lp, \
             tc.tile_pool(name="att", bufs=2) as ap_, \
             tc.tile_pool(name="st", bufs=2) as sp, \
             tc.tile_pool(name="oacc", bufs=1) as op, \
             tc.tile_pool(name="outp", bufs=2) as up, \
             tc.tile_pool(name="mm", bufs=2, space="PSUM") as mmp, \
             tc.tile_pool(name="sc", bufs=2, space="PSUM") as scp, \
             tc.tile_pool(name="pv", bufs=1, space="PSUM") as pvp, \
             tc.tile_pool(name="tr", bufs=2, space="PSUM") as trp:

            ident = cp.tile([P, P], f16, tag="ident")
            make_identity(nc, ident[:])
            gam = cp.tile([P, D], f32, tag="gam")
            bet = cp.tile([P, D], f32, tag="bet")
            g_ap = g_ext[:, :]
            nc.gpsimd.dma_start(out=gam[:], in_=bass.AP(
                tensor=g_ap.tensor, offset=g_ap.offset, ap=[[0, P], g_ap.ap[-1]]))
            b_ap = b_ext[:, :]
            nc.gpsimd.dma_start(out=bet[:], in_=bass.AP(
                tensor=b_ap.tensor, offset=b_ap.offset, ap=[[0, P], b_ap.ap[-1]]))
            eps = cp.tile([P, 1], f32, tag="eps")
            nc.vector.memset(eps[:], EPS)
            mask = cp.tile([P, 4 * D], f32, tag="mask")

            xnT = [xp.tile([P, S], f16, tag=f"xnt{j}", name=f"xnt{j}") for j in range(ND)]
            oacc = [op.tile([P, D], f16, tag=f"oacc{i}", name=f"oacc{i}") for i in range(NS)]

            def emit_ln_tile(i):
                xt = lp.tile([P, D], f32, tag="x", name="xt")
                nc.sync.dma_start(out=xt[:], in_=x_ext[i * P:(i + 1) * P, :])
                stats = lp.tile([P, 6], f32, tag="bs", name="bs")
                nc.vector.bn_stats(out=stats[:], in_=xt[:])
                mv = lp.tile([P, 2], f32, tag="mv", name="mv")
                nc.vector.bn_aggr(out=mv[:], in_=stats[:])
                sd = lp.tile([P, 1], f32, tag="sd", name="sd")
                nc.scalar.activation(out=sd[:], in_=mv[:, 1:2],
                                     func=mybir.ActivationFunctionType.Sqrt,
                                     bias=eps[:], scale=1.0, alpha=0.0)
                nc.vector.reciprocal(out=sd[:], in_=sd[:])
                nc.vector.tensor_scalar(out=xt[:], in0=xt[:],
                                        scalar1=mv[:, 0:1], scalar2=sd[:],
                                        op0=mybir.AluOpType.subtract,
                                        op1=mybir.AluOpType.mult)
                nc.vector.tensor_mul(out=xt[:], in0=xt[:], in1=gam[:])
                xh = lp.tile([P, D], f16, tag="xh", name="xh")
                nc.vector.tensor_add(out=xh[:], in0=xt[:], in1=bet[:])
                for j in range(ND):
                    tp = trp.tile([P, P], f16, tag="tr", name="tp")
                    nc.tensor.transpose(tp[:], xh[:, j * P:(j + 1) * P], ident[:])
                    nc.any.tensor_copy(out=xnT[j][:, i * P:(i + 1) * P], in_=tp[:])

            def load_w(w_ext_, h):
                wt = [wp.tile([P, U], f16, tag=f"w{j}", name=f"w{j}") for j in range(ND)]
                for j in range(ND):
                    nc.gpsimd.dma_start(
                        out=wt[j][:],
                        in_=w_ext_[h * D + j * P: h * D + (j + 1) * P, :])
                return wt

            def emit_proj_sl(wt, dst, sl):
                for u in range(NU):
                    mm = mmp.tile([P, 512], f32, tag="mm", name="mm")
                    for j in range(ND):
                        nc.tensor.matmul(mm[:],
                                         wt[j][:, u * P:(u + 1) * P],
                                         xnT[j][:, sl * 512:(sl + 1) * 512],
                                         start=(j == 0), stop=(j == ND - 1))
                    nc.any.tensor_copy(out=dst[u][:, sl * 512:(sl + 1) * 512], in_=mm[:])

            def emit_v(h):
                V = [qp.tile([P, U], f16, tag=f"v{t}", name=f"v{t}") for t in range(NS)]
                wt = [wp.tile([P, U], f16, tag=f"w{j}", name=f"w{j}") for j in range(ND)]
                for j in range(ND):
                    nc.gpsimd.dma_start(
                        out=wt[j][:],
                        in_=wv_ext[h * D + j * P: h * D + (j + 1) * P, :])
                for t in range(NS):
                    for us in range(2):
                        mm = mmp.tile([P, 512], f32, tag="mm", name="mm")
                        for j in range(ND):
                            nc.tensor.matmul(mm[:],
                                             xnT[j][:, t * P:(t + 1) * P],
                                             wt[j][:, us * 512:(us + 1) * 512],
                                             start=(j == 0), stop=(j == ND - 1))
                        nc.any.tensor_copy(out=V[t][:, us * 512:(us + 1) * 512], in_=mm[:])
                return V

            def load_wo(h):
                wo_t = [wop.tile([P, D], f16, tag=f"wo{ub}", name=f"wo{ub}") for ub in range(NU)]
                for ub in range(NU):
                    nc.gpsimd.dma_start(
                        out=wo_t[ub][:],
                        in_=wo_ext[h * U + ub * P: h * U + (ub + 1) * P, :])
                return wo_t

            def emit_scores(i, QT, KT):
                nch = i // 4 + 1
                Pt = ap_.tile([P, S], f16, tag="P", name="Pt")
                mneg = sp.tile([P, 4], f32, tag="mneg", name="mneg")
                rsum = sp.tile([P, 4], f32, tag="rsum", name="rsum")
                for c in range(nch):
                    w = (i % 4 + 1) * P if c == i // 4 else 512
                    sc = scp.tile([P, 512], f32, tag="sc", name="sc")
                    for u in range(NU):
                        nc.tensor.matmul(sc[:, 0:w],
                                         QT[u][:, i * P:(i + 1) * P],
                                         KT[u][:, c * 512:c * 512 + w],
                                         start=(u == 0), stop=(u == NU - 1))
                    if c == i // 4:
                        m = i % 4
                        nc.vector.tensor_add(out=sc[:, 0:w], in0=sc[:, 0:w],
                                             in1=mask[:, m * 512:m * 512 + w])
                    nc.vector.reduce_max(out=mneg[:, c:c + 1], in_=sc[:, 0:w],
                                         axis=mybir.AxisListType.X, negate=True)
                    nc.scalar.activation(out=Pt[:, c * 512:c * 512 + w], in_=sc[:, 0:w],
                                         func=mybir.ActivationFunctionType.Exp,
                                         bias=mneg[:, c:c + 1], scale=1.0,
                                         accum_out=rsum[:, c:c + 1])
                return Pt, mneg, rsum

            def emit_tail(h, i, Pt, mneg, rsum, V, wo_t):
                nch = i // 4 + 1
                # global softmax rescale: beta_c = exp(m_c - m_g) / Z
                mpos = sp.tile([P, 4], f32, tag="mpos", name="mpos")
                nc.vector.tensor_scalar_mul(out=mpos[:, 0:nch], in0=mneg[:, 0:nch],
                                            scalar1=-1.0)
                mgn = sp.tile([P, 1], f32, tag="mgn", name="mgn")
                nc.vector.reduce_max(out=mgn[:], in_=mpos[:, 0:nch],
                                     axis=mybir.AxisListType.X, negate=True)
                alph = sp.tile([P, 4], f32, tag="alph", name="alph")
                nc.scalar.activation(out=alph[:, 0:nch], in_=mneg[:, 0:nch],
                                     func=mybir.ActivationFunctionType.Exp,
                                     bias=mgn[:], scale=-1.0)
                pr = sp.tile([P, 4], f32, tag="pr", name="pr")
                nc.vector.tensor_mul(out=pr[:, 0:nch], in0=rsum[:, 0:nch],
                                     in1=alph[:, 0:nch])
                tot = sp.tile([P, 1], f32, tag="tot", name="tot")
                nc.vector.reduce_sum(out=tot[:], in_=pr[:, 0:nch],
                                     axis=mybir.AxisListType.X)
                nc.vector.reciprocal(out=tot[:], in_=tot[:])
                bt = sp.tile([P, 4], f32, tag="bt", name="bt")
                nc.vector.tensor_scalar_mul(out=bt[:, 0:nch], in0=alph[:, 0:nch],
                                            scalar1=tot[:])
                for c in range(nch):
                    w = (i % 4 + 1) * P if c == i // 4 else 512
                    nc.vector.tensor_scalar_mul(out=Pt[:, c * 512:c * 512 + w],
                                                in0=Pt[:, c * 512:c * 512 + w],
                                                scalar1=bt[:, c:c + 1])
                # transpose probs blocks 0..i
                pt = ap_.tile([P, S], f16, tag="pt", name="pt")
                for tb in range(i + 1):
                    tp = trp.tile([P, P], f16, tag="tr", name="tp")
                    nc.tensor.transpose(tp[:], Pt[:, tb * P:(tb + 1) * P], ident[:])
                    nc.any.tensor_copy(out=pt[:, tb * P:(tb + 1) * P], in_=tp[:])
                # probs @ V
                ht = ap_.tile([P, U], f16, tag="ht", name="ht")
                for us in range(2):
                    pv = pvp.tile([P, 512], f32, tag=f"pv{us}", name="pv")
                    for tb in range(i + 1):
                        nc.tensor.matmul(pv[:],
                                         pt[:, tb * P:(tb + 1) * P],
                                         V[tb][:, us * 512:(us + 1) * 512],
                                         start=(tb == 0), stop=(tb == i))
                    nc.any.tensor_copy(out=ht[:, us * 512:(us + 1) * 512], in_=pv[:])
                # transpose head-out blocks
                htt = ap_.tile([P, U], f16, tag="htt", name="htt")
                for ub in range(NU):
                    tp = trp.tile([P, P], f16, tag="tr", name="tp")
                    nc.tensor.transpose(tp[:], ht[:, ub * P:(ub + 1) * P], ident[:])
                    nc.any.tensor_copy(out=htt[:, ub * P:(ub + 1) * P], in_=tp[:])
                # output projection
                om = mmp.tile([P, 512], f32, tag="mm", name="om")
                for ub in range(NU):
                    nc.tensor.matmul(om[:],
                                     htt[:, ub * P:(ub + 1) * P],
                                     wo_t[ub][:],
                                     start=(ub == 0), stop=(ub == NU - 1))
                if h == 0:
                    nc.any.tensor_copy(out=oacc[i][:], in_=om[:])
                else:
                    oh = up.tile([P, D], f16, tag="oh", name="oh")
                    nc.any.tensor_copy(out=oh[:], in_=om[:])
                    of = up.tile([P, D], f32, tag="of", name="of")
                    nc.vector.tensor_add(out=of[:], in0=oh[:], in1=oacc[i][:])
                    nc.sync.dma_start(out=out_ext[i * P:(i + 1) * P, :], in_=of[:])

            # ---- LayerNorm interleaved with head-0 Q projection ----
            wt0 = load_w(wq_ext, 0)
            nc.gpsimd.dma_start(out=mask[:], in_=m_ext[:, :])
            QT0 = [qp.tile([P, S], f16, tag=f"qt{u}", name=f"qt{u}") for u in range(NU)]
            for g in range(4):
                for i in range(4 * g, 4 * g + 4):
                    emit_ln_tile(i)
                emit_proj_sl(wt0, QT0, g)

            # ---- head 0: K, V, Wout ----
            wt = load_w(wk_ext, 0)
            KT0 = [qp.tile([P, S], f16, tag=f"kt{u}", name=f"kt{u}") for u in range(NU)]
            for sl in range(4):
                emit_proj_sl(wt, KT0, sl)
            V0 = emit_v(0)
            wo_t0 = load_wo(0)

            # ---- head 0 attention, software-pipelined by one stage ----
            pend = None
            for i in range(NS):
                cur = (0, i) + emit_scores(i, QT0, KT0) + (V0, wo_t0)
                if pend is not None:
                    emit_tail(*pend)
                pend = cur

            # ---- head 1 Q/K projections fill the last softmax stall ----
            wt = load_w(wq_ext, 1)
            QT1 = [qp.tile([P, S], f16, tag=f"qt{u}", name=f"qt{u}") for u in range(NU)]
            for sl in range(4):
                emit_proj_sl(wt, QT1, sl)
            wt = load_w(wk_ext, 1)
            KT1 = [qp.tile([P, S], f16, tag=f"kt{u}", name=f"kt{u}") for u in range(NU)]
            for sl in range(4):
                emit_proj_sl(wt, KT1, sl)
            emit_tail(*pend)  # head-0 i=15: must precede V1 overwrite of v tags
            V1 = emit_v(1)
            wo_t1 = load_wo(1)

            # ---- head 1 attention ----
            pend = None
            for i in range(NS):
                cur = (1, i) + emit_scores(i, QT1, KT1) + (V1, wo_t1)
                if pend is not None:
                    emit_tail(*pend)
                pend = cur
            emit_tail(*pend)
    return nc


_NC = None


def _get_nc():
    global _NC
    if _NC is None:
        _NC = _build()
    return _NC


def _mask_np():
    r = np.arange(P)[:, None]
    j = np.arange(D)[None, :]
    m = np.empty((P, 4 * D), np.float32)
    for k in range(4):
        m[:, k * D:(k + 1) * D] = np.where(j <= k * P + r, 0.0, NEG)
    return m


def _run(inputs, trace=False):
    x = np.asarray(inputs["x"], dtype=np.float32)          # [4, 2048, 512]
    gamma = np.asarray(inputs["gamma"], dtype=np.float32).reshape(1, D)
    beta = np.asarray(inputs["beta"], dtype=np.float32).reshape(1, D)
    Wq = np.asarray(inputs["Wq"], dtype=np.float32)        # [4, 512, 1024]
    Wk = np.asarray(inputs["Wk"], dtype=np.float32)
    Wv = np.asarray(inputs["Wv"], dtype=np.float32)
    Wout = np.asarray(inputs["Wout"], dtype=np.float32)    # [4096, 512]
    mask = _mask_np()

    in_maps = []
    for c in range(8):
        b, hp = c // 2, c % 2
        in_maps.append({
            "x": np.ascontiguousarray(x[b]),
            "gamma": gamma,
            "beta": beta,
            "wq": np.ascontiguousarray(Wq[2 * hp:2 * hp + 2].reshape(2 * D, U)).astype(np.float16),
            "wk": np.ascontiguousarray(Wk[2 * hp:2 * hp + 2].reshape(2 * D, U)).astype(np.float16),
            "wv": np.ascontiguousarray(Wv[2 * hp:2 * hp + 2].reshape(2 * D, U)).astype(np.float16),
            "wo": np.ascontiguousarray(Wout[2 * hp * U:(2 * hp + 2) * U]).astype(np.float16),
            "mask": mask,
        })
    res = run_bass_kernel_spmd(_get_nc(), in_maps, list(range(8)), trace=trace)
    out = np.empty((4, S, D), np.float32)
    for b in range(4):
        out[b] = res.results[2 * b]["out"] + res.results[2 * b + 1]["out"]
    return out, res


def kernel(**inputs):
    out, _ = _run(inputs, trace=False)
    return out


# revision 20
# speedup vs baseline: 1.1231x; 1.0044x over previous
import numpy as np
import concourse.bass as bass
import concourse.tile as tile
from concourse import mybir
from concourse.bass_utils import run_bass_kernel_spmd
from concourse.masks import make_identity

P = 128
S = 2048
D = 512
U = 1024
NS = S // P      # 16 s-tiles
ND = D // P      # 4 d-blocks
NU = U // P      # 8 u-blocks
NEG = -60000.0
EPS = 1e-6


def _patched_drain_and_barrier(self, tick_clock, wait_clock):
    nc = self.nc
    probe = nc.sync.nop(nofuse=True, hint="drain_waits_probe")
    wait_clock.add_sem_waits(probe.ins, tile.ScopedClock({None: tick_clock.global_clock}))
    si = probe.ins.sync_info
    waits = list(si.on_wait) if si is not None else []
    assert self.sems is not None
    handles = {h.name: h for h in self.sems.allocated().values()}
    if len(waits) > 1:
        import bass_rust
        probe.ins.sync_info = bass_rust.SyncInfo(on_wait=waits[:1], on_update=[])
        for w in waits[1:]:
            h = handles.get(w.ant_name)
            assert h is not None, (w.ant_name, list(handles))
            nc.sync.wait_ge(h, w.wait_value)
    nc.sync.drain()
    nc.all_engine_barrier()
    popped = nc._tile_sem_poison_stack.pop()
    assert popped is self._sem_poison
    nc.clear_and_free_semaphores(list(self.sems.allocated().values()))
    nc.all_engine_barrier()


tile.TileContext._drain_and_barrier = _patched_drain_and_barrier

# The walrus backend in this toolchain rejects instructions carrying more
# than one semaphore wait ("Too many sync wait commands"). Split excess
# waits onto single-wait NoOp carriers on the same engine, which execute
# in order ahead of the real instruction.
_MAXW = 1
_orig_lower_ordered = tile.TileContext._lower_ordered_insts


def _patched_lower_ordered(self, ordered):
    nc = self.nc
    for insts in ordered.values():
        out = []
        for inst in insts:
            si = getattr(inst, "sync_info", None)
            eng = getattr(inst, "engine", None)
            if (si is not None and si.on_wait and len(si.on_wait) > _MAXW
                    and eng is not None
                    and not type(inst).__name__.startswith("BassTile")):
                waits = list(si.on_wait)
                for w in waits[:-_MAXW]:
                    out.append(mybir.InstNoOp(
                        name=nc.get_next_instruction_name(),
                        engine=eng,
                        ins=[],
                        outs=[],
                        bass_nofuse=True,
                        sync_info=mybir.SyncInfo(on_wait=[w], on_update=[]),
                    ))
                inst.sync_info = mybir.SyncInfo(
                    on_wait=waits[-_MAXW:], on_update=list(si.on_update))
            out.append(inst)
        insts[:] = out
    return _orig_lower_ordered(self, ordered)


tile.TileContext._lower_ordered_insts = _patched_lower_ordered

f32 = mybir.dt.float32
f16 = mybir.dt.float16


def _build():
    nc = bass.Bass()
    x_ext = nc.declare_dram_parameter("x", [S, D], f32, isOutput=False)
    bq_ext = nc.declare_dram_parameter("bq", [P, 2 * NU], f32, isOutput=False)
    wq_ext = nc.declare_dram_parameter("wq", [2 * D, U], f16, isOutput=False)
    wk_ext = nc.declare_dram_parameter("wk", [2 * D, U], f16, isOutput=False)
    wv_ext = nc.declare_dram_parameter("wv", [2 * D, U], f16, isOutput=False)
    wo_ext = nc.declare_dram_parameter("wo", [2 * U, D], f16, isOutput=False)
    m_ext = nc.declare_dram_parameter("mask", [P, 4 * D], f32, isOutput=False)
    out_ext = nc.declare_dram_parameter("out", [S, D], f32, isOutput=True)

    with tile.TileContext(nc) as tc:
        with tc.tile_pool(name="const", bufs=1) as cp, \
             tc.tile_pool(name="xnt", bufs=1) as xp, \
             tc.tile_pool(name="wp", bufs=1) as wp, \
             tc.tile_pool(name="wop", bufs=1) as wop, \
             tc.tile_pool(name="qkv", bufs=1) as qp, \
             tc.tile_pool(name="ln", bufs=2) as lp, \
             tc.tile_pool(name="att", bufs=2) as ap_, \
             tc.tile_pool(name="st", bufs=2) as sp, \
             tc.tile_pool(name="oacc", bufs=1) as op, \
             tc.tile_pool(name="outp", bufs=2) as up, \
             tc.tile_pool(name="mm", bufs=2, space="PSUM") as mmp, \
             tc.tile_pool(name="sc", bufs=2, space="PSUM") as scp, \
             tc.tile_pool(name="pv", bufs=1, space="PSUM") as pvp, \
             tc.tile_pool(name="tr", bufs=2, space="PSUM") as trp:

            ident = cp.tile([P, P], f16, tag="ident")
            make_identity(nc, ident[:])
            bqt = cp.tile([P, 2 * NU], f32, tag="bqt")
            nc.sync.dma_start(out=bqt[:], in_=bq_ext[:, :])
            eps = cp.tile([P, 1], f32, tag="eps")
            nc.vector.memset(eps[:], EPS)
            mask = cp.tile([P, 4 * D], f32, tag="mask")

            xnT = [xp.tile([P, S], f16, tag=f"xnt{j}", name=f"xnt{j}") for j in range(ND)]
            oacc = [op.tile([P, D], f16, tag=f"oacc{i}", name=f"oacc{i}") for i in range(NS)]

            def emit_ln_tile(i):
                xt = lp.tile([P, D], f32, tag="x", name="xt")
                nc.sync.dma_start(out=xt[:], in_=x_ext[i * P:(i + 1) * P, :])
                stats = lp.tile([P, 6], f32, tag="bs", name="bs")
                nc.vector.bn_stats(out=stats[:], in_=xt[:])
                mv = lp.tile([P, 2], f32, tag="mv", name="mv")
                nc.vector.bn_aggr(out=mv[:], in_=stats[:])
                sd = lp.tile([P, 1], f32, tag="sd", name="sd")
                nc.scalar.activation(out=sd[:], in_=mv[:, 1:2],
                                     func=mybir.ActivationFunctionType.Sqrt,
                                     bias=eps[:], scale=1.0, alpha=0.0)
                nc.vector.reciprocal(out=sd[:], in_=sd[:])
                xh = lp.tile([P, D], f16, tag="xh", name="xh")
                nc.vector.tensor_scalar(out=xh[:], in0=xt[:],
                                        scalar1=mv[:, 0:1], scalar2=sd[:],
                                        op0=mybir.AluOpType.subtract,
                                        op1=mybir.AluOpType.mult)
                for j in range(ND):
                    tp = trp.tile([P, P], f16, tag="tr", name="tp")
                    nc.tensor.transpose(tp[:], xh[:, j * P:(j + 1) * P], ident[:])
                    nc.any.tensor_copy(out=xnT[j][:, i * P:(i + 1) * P], in_=tp[:])

            def load_w(w_ext_, h):
                wt = [wp.tile([P, U], f16, tag=f"w{j}", name=f"w{j}") for j in range(ND)]
                for j in range(ND):
                    nc.gpsimd.dma_start(
                        out=wt[j][:],
                        in_=w_ext_[h * D + j * P: h * D + (j + 1) * P, :])
                return wt

            def emit_proj_sl(wt, dst, sl, bcol=None):
                for u in range(NU):
                    mm = mmp.tile([P, 512], f32, tag="mm", name="mm")
                    for j in range(ND):
                        nc.tensor.matmul(mm[:],
                                         wt[j][:, u * P:(u + 1) * P],
                                         xnT[j][:, sl * 512:(sl + 1) * 512],
                                         start=(j == 0), stop=(j == ND - 1))
                    if bcol is None:
                        nc.any.tensor_copy(out=dst[u][:, sl * 512:(sl + 1) * 512], in_=mm[:])
                    else:
                        nc.any.tensor_scalar_add(out=dst[u][:, sl * 512:(sl + 1) * 512],
                                                 in0=mm[:],
                                                 scalar1=bqt[:, bcol + u:bcol + u + 1])

            def emit_v(h):
                V = [qp.tile([P, U], f16, tag=f"v{t}", name=f"v{t}") for t in range(NS)]
                wt = [wp.tile([P, U], f16, tag=f"w{j}", name=f"w{j}") for j in range(ND)]
                for j in range(ND):
                    nc.gpsimd.dma_start(
                        out=wt[j][:],
                        in_=wv_ext[h * D + j * P: h * D + (j + 1) * P, :])
                for t in range(NS):
                    for us in range(2):
                        mm = mmp.tile([P, 512], f32, tag="mm", name="mm")
                        for j in range(ND):
                            nc.tensor.matmul(mm[:],
                                             xnT[j][:, t * P:(t + 1) * P],
                                             wt[j][:, us * 512:(us + 1) * 512],
                                             start=(j == 0), stop=(j == ND - 1))
                        nc.any.tensor_copy(out=V[t][:, us * 512:(us + 1) * 512], in_=mm[:])
                return V

            def load_wo(h):
                wo_t = [wop.tile([P, D], f16, tag=f"wo{ub}", name=f"wo{ub}") for ub in range(NU)]
                for ub in range(NU):
                    nc.gpsimd.dma_start(
                        out=wo_t[ub][:],
                        in_=wo_ext[h * U + ub * P: h * U + (ub + 1) * P, :])
                return wo_t

            def emit_scores(i, QT, KT):
                nch = i // 4 + 1
                Pt = ap_.tile([P, S], f16, tag="P", name="Pt")
                mneg = sp.tile([P, 4], f32, tag="mneg", name="mneg")
                rsum = sp.tile([P, 4], f32, tag="rsum", name="rsum")
                for c in range(nch):
                    w = (i % 4 + 1) * P if c == i // 4 else 512
                    sc = scp.tile([P, 512], f32, tag="sc", name="sc")
                    for u in range(NU):
                        nc.tensor.matmul(sc[:, 0:w],
                                         QT[u][:, i * P:(i + 1) * P],
                                         KT[u][:, c * 512:c * 512 + w],
                                         start=(u == 0), stop=(u == NU - 1))
                    if c == i // 4:
                        m = i % 4
                        nc.vector.tensor_add(out=sc[:, 0:w], in0=sc[:, 0:w],
                                             in1=mask[:, m * 512:m * 512 + w])
                    nc.vector.reduce_max(out=mneg[:, c:c + 1], in_=sc[:, 0:w],
                                         axis=mybir.AxisListType.X, negate=True)
                    nc.scalar.activation(out=Pt[:, c * 512:c * 512 + w], in_=sc[:, 0:w],
                                         func=mybir.ActivationFunctionType.Exp,
                                         bias=mneg[:, c:c + 1], scale=1.0,
                                         accum_out=rsum[:, c:c + 1])
                return Pt, mneg, rsum

            def emit_tail(h, i, Pt, mneg, rsum, V, wo_t):
                nch = i // 4 + 1
                # global softmax rescale: beta_c = exp(m_c - m_g) / Z
                mpos = sp.tile([P, 4], f32, tag="mpos", name="mpos")
                nc.vector.tensor_scalar_mul(out=mpos[:, 0:nch], in0=mneg[:, 0:nch],
                                            scalar1=-1.0)
                mgn = sp.tile([P, 1], f32, tag="mgn", name="mgn")
                nc.vector.reduce_max(out=mgn[:], in_=mpos[:, 0:nch],
                                     axis=mybir.AxisListType.X, negate=True)
                alph = sp.tile([P, 4], f32, tag="alph", name="alph")
                nc.scalar.activation(out=alph[:, 0:nch], in_=mneg[:, 0:nch],
                                     func=mybir.ActivationFunctionType.Exp,
                                     bias=mgn[:], scale=-1.0)
                pr = sp.tile([P, 4], f32, tag="pr", name="pr")
                nc.vector.tensor_mul(out=pr[:, 0:nch], in0=rsum[:, 0:nch],
                                     in1=alph[:, 0:nch])
                tot = sp.tile([P, 1], f32, tag="tot", name="tot")
                nc.vector.reduce_sum(out=tot[:], in_=pr[:, 0:nch],
                                     axis=mybir.AxisListType.X)
                nc.vector.reciprocal(out=tot[:], in_=tot[:])
                bt = sp.tile([P, 4], f32, tag="bt", name="bt")
                nc.vector.tensor_scalar_mul(out=bt[:, 0:nch], in0=alph[:, 0:nch],
                                            scalar1=tot[:])
                for c in range(nch):
                    w = (i % 4 + 1) * P if c == i // 4 else 512
                    nc.vector.tensor_scalar_mul(out=Pt[:, c * 512:c * 512 + w],
                                                in0=Pt[:, c * 512:c * 512 + w],
                                                scalar1=bt[:, c:c + 1])
                # transpose probs blocks 0..i
                pt = ap_.tile([P, S], f16, tag="pt", name="pt")
                for tb in range(i + 1):
                    tp = trp.tile([P, P], f16, tag="tr", name="tp")
                    nc.tensor.transpose(tp[:], Pt[:, tb * P:(tb + 1) * P], ident[:])
                    nc.any.tensor_copy(out=pt[:, tb * P:(tb + 1) * P], in_=tp[:])
                # probs @ V
                ht = ap_.tile([P, U], f16, tag="ht", name="ht")
                for us in range(2):
                    pv = pvp.tile([P, 512], f32, tag=f"pv{us}", name="pv")
                    for tb in range(i + 1):
                        nc.tensor.matmul(pv[:],
                                         pt[:, tb * P:(tb + 1) * P],
                                         V[tb][:, us * 512:(us + 1) * 512],
                                         start=(tb == 0), stop=(tb == i))
                    nc.any.tensor_copy(out=ht[:, us * 512:(us + 1) * 512], in_=pv[:])
                # transpose head-out blocks
                htt = ap_.tile([P, U], f16, tag="htt", name="htt")
                for ub in range(NU):
                    tp = trp.tile([P, P], f16, tag="tr", name="tp")
                    nc.tensor.transpose(tp[:], ht[:, ub * P:(ub + 1) * P], ident[:])
                    nc.any.tensor_copy(out=htt[:, ub * P:(ub + 1) * P], in_=tp[:])
                # output projection
                om = mmp.tile([P, 512], f32, tag="mm", name="om")
                for ub in range(NU):
                    nc.tensor.matmul(om[:],
                                     htt[:, ub * P:(ub + 1) * P],
                                     wo_t[ub][:],
                                     start=(ub == 0), stop=(ub == NU - 1))
                if h == 0:
                    nc.any.tensor_copy(out=oacc[i][:], in_=om[:])
                else:
                    oh = up.tile([P, D], f16, tag="oh", name="oh")
                    nc.any.tensor_copy(out=oh[:], in_=om[:])
                    of = up.tile([P, D], f32, tag="of", name="of")
                    nc.vector.tensor_add(out=of[:], in0=oh[:], in1=oacc[i][:])
                    nc.sync.dma_start(out=out_ext[i * P:(i + 1) * P, :], in_=of[:])

            # ---- LayerNorm interleaved with head-0 Q projection ----
            wt0 = load_w(wq_ext, 0)
            nc.gpsimd.dma_start(out=mask[:], in_=m_ext[:, :])
            QT0 = [qp.tile([P, S], f16, tag=f"qt{u}", name=f"qt{u}") for u in range(NU)]
            for g in range(4):
                for i in range(4 * g, 4 * g + 4):
                    emit_ln_tile(i)
                emit_proj_sl(wt0, QT0, g, bcol=0)

            # ---- head 0: K, V, Wout ----
            wt = load_w(wk_ext, 0)
            KT0 = [qp.tile([P, S], f16, tag=f"kt{u}", name=f"kt{u}") for u in range(NU)]
            for sl in range(4):
                emit_proj_sl(wt, KT0, sl)
            V0 = emit_v(0)
            wo_t0 = load_wo(0)

            # ---- head 0 attention, software-pipelined by one stage ----
            pend = None
            for i in range(NS):
                cur = (0, i) + emit_scores(i, QT0, KT0) + (V0, wo_t0)
                if pend is not None:
                    emit_tail(*pend)
                pend = cur

            # ---- head 1 Q/K projections fill the last softmax stall ----
            wt = load_w(wq_ext, 1)
            QT1 = [qp.tile([P, S], f16, tag=f"qt{u}", name=f"qt{u}") for u in range(NU)]
            for sl in range(4):
                emit_proj_sl(wt, QT1, sl, bcol=NU)
            wt = load_w(wk_ext, 1)
            KT1 = [qp.tile([P, S], f16, tag=f"kt{u}", name=f"kt{u}") for u in range(NU)]
            for sl in range(4):
                emit_proj_sl(wt, KT1, sl)
            emit_tail(*pend)  # head-0 i=15: must precede V1 overwrite of v tags
            V1 = emit_v(1)
            wo_t1 = load_wo(1)

            # ---- head 1 attention ----
            pend = None
            for i in range(NS):
                cur = (1, i) + emit_scores(i, QT1, KT1) + (V1, wo_t1)
                if pend is not None:
                    emit_tail(*pend)
                pend = cur
            emit_tail(*pend)
    return nc


_NC = None


def _get_nc():
    global _NC
    if _NC is None:
        _NC = _build()
    return _NC


def _mask_np():
    r = np.arange(P)[:, None]
    j = np.arange(D)[None, :]
    m = np.empty((P, 4 * D), np.float32)
    for k in range(4):
        m[:, k * D:(k + 1) * D] = np.where(j <= k * P + r, 0.0, NEG)
    return m


def _run(inputs, trace=False):
    x = np.asarray(inputs["x"], dtype=np.float32)          # [4, 2048, 512]
    gamma = np.asarray(inputs["gamma"], dtype=np.float32).reshape(D)
    beta = np.asarray(inputs["beta"], dtype=np.float32).reshape(D)
    Wq = np.asarray(inputs["Wq"], dtype=np.float32)        # [4, 512, 1024]
    Wk = np.asarray(inputs["Wk"], dtype=np.float32)
    Wv = np.asarray(inputs["Wv"], dtype=np.float32)
    Wout = np.asarray(inputs["Wout"], dtype=np.float32)    # [4096, 512]
    mask = _mask_np()

    # fold LN gamma into projection weights; beta terms:
    #  - K bias shifts each score row by a constant -> cancels in softmax
    #  - V bias passes through softmax (rows sum to 1) -> host-side constant
    #  - Q bias added in-kernel during psum evacuation
    Wqf = Wq * gamma[None, :, None]
    Wkf = Wk * gamma[None, :, None]
    Wvf = Wv * gamma[None, :, None]
    bq_all = np.einsum("d,hdu->hu", beta, Wq)              # [4, 1024]
    bv_all = np.einsum("d,hdu->hu", beta, Wv)              # [4, 1024]
    cvec = np.zeros(D, np.float32)
    for h in range(4):
        cvec += bv_all[h] @ Wout[h * U:(h + 1) * U]

    in_maps = []
    for c in range(8):
        b, hp = c // 2, c % 2
        bq = bq_all[2 * hp:2 * hp + 2].reshape(2, NU, P).transpose(2, 0, 1).reshape(P, 2 * NU)
        in_maps.append({
            "x": np.ascontiguousarray(x[b]),
            "bq": np.ascontiguousarray(bq),
            "wq": np.ascontiguousarray(Wqf[2 * hp:2 * hp + 2].reshape(2 * D, U)).astype(np.float16),
            "wk": np.ascontiguousarray(Wkf[2 * hp:2 * hp + 2].reshape(2 * D, U)).astype(np.float16),
            "wv": np.ascontiguousarray(Wvf[2 * hp:2 * hp + 2].reshape(2 * D, U)).astype(np.float16),
            "wo": np.ascontiguousarray(Wout[2 * hp * U:(2 * hp + 2) * U]).astype(np.float16),
            "mask": mask,
        })
    res = run_bass_kernel_spmd(_get_nc(), in_maps, list(range(8)), trace=trace)
    out = np.empty((4, S, D), np.float32)
    for b in range(4):
        out[b] = res.results[2 * b]["out"] + res.results[2 * b + 1]["out"] + cvec[None, :]
    return out, res


def kernel(**inputs):
    out, _ = _run(inputs, trace=False)
    return out


# revision 26
# speedup vs baseline: 1.1446x; 1.0192x over previous
import numpy as np
import concourse.bass as bass
import concourse.tile as tile
from concourse import mybir
from concourse.bass_utils import run_bass_kernel_spmd
from concourse.masks import make_identity

P = 128
S = 2048
D = 512
U = 1024
NS = S // P      # 16 s-tiles
ND = D // P      # 4 d-blocks
NU = U // P      # 8 u-blocks
NEG = -60000.0
EPS = 1e-6


def _patched_drain_and_barrier(self, tick_clock, wait_clock):
    nc = self.nc
    probe = nc.sync.nop(nofuse=True, hint="drain_waits_probe")
    wait_clock.add_sem_waits(probe.ins, tile.ScopedClock({None: tick_clock.global_clock}))
    si = probe.ins.sync_info
    waits = list(si.on_wait) if si is not None else []
    assert self.sems is not None
    handles = {h.name: h for h in self.sems.allocated().values()}
    if len(waits) > 1:
        import bass_rust
        probe.ins.sync_info = bass_rust.SyncInfo(on_wait=waits[:1], on_update=[])
        for w in waits[1:]:
            h = handles.get(w.ant_name)
            assert h is not None, (w.ant_name, list(handles))
            nc.sync.wait_ge(h, w.wait_value)
    nc.sync.drain()
    nc.all_engine_barrier()
    popped = nc._tile_sem_poison_stack.pop()
    assert popped is self._sem_poison
    nc.clear_and_free_semaphores(list(self.sems.allocated().values()))
    nc.all_engine_barrier()


tile.TileContext._drain_and_barrier = _patched_drain_and_barrier

# The walrus backend in this toolchain rejects instructions carrying more
# than one semaphore wait ("Too many sync wait commands"). Split excess
# waits onto single-wait NoOp carriers on the same engine, which execute
# in order ahead of the real instruction.
_MAXW = 1
_orig_lower_ordered = tile.TileContext._lower_ordered_insts


def _patched_lower_ordered(self, ordered):
    nc = self.nc
    for insts in ordered.values():
        out = []
        for inst in insts:
            si = getattr(inst, "sync_info", None)
            eng = getattr(inst, "engine", None)
            if (si is not None and si.on_wait and len(si.on_wait) > _MAXW
                    and eng is not None
                    and not type(inst).__name__.startswith("BassTile")):
                waits = list(si.on_wait)
                for w in waits[:-_MAXW]:
                    out.append(mybir.InstNoOp(
                        name=nc.get_next_instruction_name(),
                        engine=eng,
                        ins=[],
                        outs=[],
                        bass_nofuse=True,
                        sync_info=mybir.SyncInfo(on_wait=[w], on_update=[]),
                    ))
                inst.sync_info = mybir.SyncInfo(
                    on_wait=waits[-_MAXW:], on_update=list(si.on_update))
            out.append(inst)
        insts[:] = out
    return _orig_lower_ordered(self, ordered)


tile.TileContext._lower_ordered_insts = _patched_lower_ordered

f32 = mybir.dt.float32
f16 = mybir.dt.float16


def _build():
    nc = bass.Bass()
    x_ext = nc.declare_dram_parameter("x", [S, D], f32, isOutput=False)
    bq_ext = nc.declare_dram_parameter("bq", [P, 2 * NU], f32, isOutput=False)
    wq_ext = nc.declare_dram_parameter("wq", [2 * D, U], f16, isOutput=False)
    wk_ext = nc.declare_dram_parameter("wk", [2 * D, U], f16, isOutput=False)
    wv_ext = nc.declare_dram_parameter("wv", [2 * D, U], f16, isOutput=False)
    wo_ext = nc.declare_dram_parameter("wo", [2 * U, D], f16, isOutput=False)
    m_ext = nc.declare_dram_parameter("mask", [P, 4 * D], f32, isOutput=False)
    out_ext = nc.declare_dram_parameter("out", [S, D], f32, isOutput=True)

    with tile.TileContext(nc) as tc:
        with tc.tile_pool(name="const", bufs=1) as cp, \
             tc.tile_pool(name="xnt", bufs=1) as xp, \
             tc.tile_pool(name="wp", bufs=1) as wp, \
             tc.tile_pool(name="wop", bufs=1) as wop, \
             tc.tile_pool(name="qkv", bufs=1) as qp, \
             tc.tile_pool(name="ln", bufs=2) as lp, \
             tc.tile_pool(name="att", bufs=2) as ap_, \
             tc.tile_pool(name="st", bufs=2) as sp, \
             tc.tile_pool(name="oacc", bufs=1) as op, \
             tc.tile_pool(name="outp", bufs=2) as up, \
             tc.tile_pool(name="mm", bufs=2, space="PSUM") as mmp, \
             tc.tile_pool(name="sc", bufs=2, space="PSUM") as scp, \
             tc.tile_pool(name="pv", bufs=1, space="PSUM") as pvp, \
             tc.tile_pool(name="tr", bufs=2, space="PSUM") as trp:

            ident = cp.tile([P, P], f16, tag="ident")
            make_identity(nc, ident[:])
            bqt = cp.tile([P, 2 * NU], f32, tag="bqt")
            nc.sync.dma_start(out=bqt[:], in_=bq_ext[:, :])
            eps = cp.tile([P, 1], f32, tag="eps")
            nc.vector.memset(eps[:], EPS)
            mask = cp.tile([P, 4 * D], f32, tag="mask")

            xnT = [xp.tile([P, S], f16, tag=f"xnt{j}", name=f"xnt{j}") for j in range(ND)]
            oacc = [op.tile([P, D], f16, tag=f"oacc{i}", name=f"oacc{i}") for i in range(NS)]

            dmaq = [nc.sync, nc.scalar]

            def emit_ln_tile(i):
                xt = lp.tile([P, D], f32, tag="x", name="xt")
                dmaq[i % 2].dma_start(out=xt[:], in_=x_ext[i * P:(i + 1) * P, :])
                stats = lp.tile([P, 6], f32, tag="bs", name="bs")
                nc.vector.bn_stats(out=stats[:], in_=xt[:])
                mv = lp.tile([P, 2], f32, tag="mv", name="mv")
                nc.vector.bn_aggr(out=mv[:], in_=stats[:])
                sd = lp.tile([P, 1], f32, tag="sd", name="sd")
                nc.scalar.activation(out=sd[:], in_=mv[:, 1:2],
                                     func=mybir.ActivationFunctionType.Sqrt,
                                     bias=eps[:], scale=1.0, alpha=0.0)
                nc.vector.reciprocal(out=sd[:], in_=sd[:])
                xh = lp.tile([P, D], f16, tag="xh", name="xh")
                nc.vector.tensor_scalar(out=xh[:], in0=xt[:],
                                        scalar1=mv[:, 0:1], scalar2=sd[:],
                                        op0=mybir.AluOpType.subtract,
                                        op1=mybir.AluOpType.mult)
                for j in range(ND):
                    tp = trp.tile([P, P], f16, tag="tr", name="tp")
                    nc.tensor.transpose(tp[:], xh[:, j * P:(j + 1) * P], ident[:])
                    nc.any.tensor_copy(out=xnT[j][:, i * P:(i + 1) * P], in_=tp[:])

            def load_w(w_ext_, h):
                wt = [wp.tile([P, U], f16, tag=f"w{j}", name=f"w{j}") for j in range(ND)]
                for j in range(ND):
                    nc.gpsimd.dma_start(
                        out=wt[j][:],
                        in_=w_ext_[h * D + j * P: h * D + (j + 1) * P, :])
                return wt

            def emit_proj_sl(wt, dst, sl, bcol=None):
                for u in range(NU):
                    mm = mmp.tile([P, 512], f32, tag="mm", name="mm")
                    for j in range(ND):
                        nc.tensor.matmul(mm[:],
                                         wt[j][:, u * P:(u + 1) * P],
                                         xnT[j][:, sl * 512:(sl + 1) * 512],
                                         start=(j == 0), stop=(j == ND - 1))
                    if bcol is None:
                        nc.any.tensor_copy(out=dst[u][:, sl * 512:(sl + 1) * 512], in_=mm[:])
                    else:
                        nc.any.tensor_scalar_add(out=dst[u][:, sl * 512:(sl + 1) * 512],
                                                 in0=mm[:],
                                                 scalar1=bqt[:, bcol + u:bcol + u + 1])

            def emit_v_prep(h):
                V = [qp.tile([P, U], f16, tag=f"v{t}", name=f"v{t}") for t in range(NS)]
                wt = [wp.tile([P, U], f16, tag=f"w{j}", name=f"w{j}") for j in range(ND)]
                for j in range(ND):
                    nc.gpsimd.dma_start(
                        out=wt[j][:],
                        in_=wv_ext[h * D + j * P: h * D + (j + 1) * P, :])
                return V, wt

            def emit_v_tile(V, wt, t):
                for us in range(2):
                    mm = mmp.tile([P, 512], f32, tag="mm", name="mm")
                    for j in range(ND):
                        nc.tensor.matmul(mm[:],
                                         xnT[j][:, t * P:(t + 1) * P],
                                         wt[j][:, us * 512:(us + 1) * 512],
                                         start=(j == 0), stop=(j == ND - 1))
                    nc.any.tensor_copy(out=V[t][:, us * 512:(us + 1) * 512], in_=mm[:])

            def load_wo(h):
                wo_t = [wop.tile([P, D], f16, tag=f"wo{ub}", name=f"wo{ub}") for ub in range(NU)]
                for ub in range(NU):
                    nc.gpsimd.dma_start(
                        out=wo_t[ub][:],
                        in_=wo_ext[h * U + ub * P: h * U + (ub + 1) * P, :])
                return wo_t

            def emit_scores(i, QT, KT):
                nch = i // 4 + 1
                Pt = ap_.tile([P, S], f16, tag="P", name="Pt")
                mneg = sp.tile([P, 4], f32, tag="mneg", name="mneg")
                rsum = sp.tile([P, 4], f32, tag="rsum", name="rsum")
                for c in range(nch):
                    w = (i % 4 + 1) * P if c == i // 4 else 512
                    sc = scp.tile([P, 512], f32, tag="sc", name="sc")
                    for u in range(NU):
                        nc.tensor.matmul(sc[:, 0:w],
                                         QT[u][:, i * P:(i + 1) * P],
                                         KT[u][:, c * 512:c * 512 + w],
                                         start=(u == 0), stop=(u == NU - 1))
                    if c == i // 4:
                        m = i % 4
                        nc.vector.tensor_add(out=sc[:, 0:w], in0=sc[:, 0:w],
                                             in1=mask[:, m * 512:m * 512 + w])
                    nc.vector.reduce_max(out=mneg[:, c:c + 1], in_=sc[:, 0:w],
                                         axis=mybir.AxisListType.X, negate=True)
                    nc.scalar.activation(out=Pt[:, c * 512:c * 512 + w], in_=sc[:, 0:w],
                                         func=mybir.ActivationFunctionType.Exp,
                                         bias=mneg[:, c:c + 1], scale=1.0,
                                         accum_out=rsum[:, c:c + 1])
                return Pt, mneg, rsum

            def emit_tail(h, i, Pt, mneg, rsum, V, wo_t):
                nch = i // 4 + 1
                # global softmax rescale: beta_c = exp(m_c - m_g) / Z
                mpos = sp.tile([P, 4], f32, tag="mpos", name="mpos")
                nc.vector.tensor_scalar_mul(out=mpos[:, 0:nch], in0=mneg[:, 0:nch],
                                            scalar1=-1.0)
                mgn = sp.tile([P, 1], f32, tag="mgn", name="mgn")
                nc.vector.reduce_max(out=mgn[:], in_=mpos[:, 0:nch],
                                     axis=mybir.AxisListType.X, negate=True)
                alph = sp.tile([P, 4], f32, tag="alph", name="alph")
                nc.scalar.activation(out=alph[:, 0:nch], in_=mneg[:, 0:nch],
                                     func=mybir.ActivationFunctionType.Exp,
                                     bias=mgn[:], scale=-1.0)
                pr = sp.tile([P, 4], f32, tag="pr", name="pr")
                nc.vector.tensor_mul(out=pr[:, 0:nch], in0=rsum[:, 0:nch],
                                     in1=alph[:, 0:nch])
                tot = sp.tile([P, 1], f32, tag="tot", name="tot")
                nc.vector.reduce_sum(out=tot[:], in_=pr[:, 0:nch],
                                     axis=mybir.AxisListType.X)
                nc.vector.reciprocal(out=tot[:], in_=tot[:])
                bt = sp.tile([P, 4], f32, tag="bt", name="bt")
                nc.vector.tensor_scalar_mul(out=bt[:, 0:nch], in0=alph[:, 0:nch],
                                            scalar1=tot[:])
                for c in range(nch):
                    w = (i % 4 + 1) * P if c == i // 4 else 512
                    nc.vector.tensor_scalar_mul(out=Pt[:, c * 512:c * 512 + w],
                                                in0=Pt[:, c * 512:c * 512 + w],
                                                scalar1=bt[:, c:c + 1])
                # transpose probs blocks 0..i
                pt = ap_.tile([P, S], f16, tag="pt", name="pt")
                for tb in range(i + 1):
                    tp = trp.tile([P, P], f16, tag="tr", name="tp")
                    nc.tensor.transpose(tp[:], Pt[:, tb * P:(tb + 1) * P], ident[:])
                    nc.any.tensor_copy(out=pt[:, tb * P:(tb + 1) * P], in_=tp[:])
                # probs @ V
                ht = ap_.tile([P, U], f16, tag="ht", name="ht")
                for us in range(2):
                    pv = pvp.tile([P, 512], f32, tag=f"pv{us}", name="pv")
                    for tb in range(i + 1):
                        nc.tensor.matmul(pv[:],
                                         pt[:, tb * P:(tb + 1) * P],
                                         V[tb][:, us * 512:(us + 1) * 512],
                                         start=(tb == 0), stop=(tb == i))
                    nc.any.tensor_copy(out=ht[:, us * 512:(us + 1) * 512], in_=pv[:])
                # transpose head-out blocks
                htt = ap_.tile([P, U], f16, tag="htt", name="htt")
                for ub in range(NU):
                    tp = trp.tile([P, P], f16, tag="tr", name="tp")
                    nc.tensor.transpose(tp[:], ht[:, ub * P:(ub + 1) * P], ident[:])
                    nc.any.tensor_copy(out=htt[:, ub * P:(ub + 1) * P], in_=tp[:])
                # output projection
                om = mmp.tile([P, 512], f32, tag="mm", name="om")
                for ub in range(NU):
                    nc.tensor.matmul(om[:],
                                     htt[:, ub * P:(ub + 1) * P],
                                     wo_t[ub][:],
                                     start=(ub == 0), stop=(ub == NU - 1))
                if h == 0:
                    nc.any.tensor_copy(out=oacc[i][:], in_=om[:])
                else:
                    oh = up.tile([P, D], f16, tag="oh", name="oh")
                    nc.any.tensor_copy(out=oh[:], in_=om[:])
                    of = up.tile([P, D], f32, tag="of", name="of")
                    nc.vector.tensor_add(out=of[:], in0=oh[:], in1=oacc[i][:])
                    nc.sync.dma_start(out=out_ext[i * P:(i + 1) * P, :], in_=of[:])

            # ---- LayerNorm interleaved with head-0 Q projection ----
            wt0 = load_w(wq_ext, 0)
            nc.gpsimd.dma_start(out=mask[:], in_=m_ext[:, :])
            QT0 = [qp.tile([P, S], f16, tag=f"qt{u}", name=f"qt{u}") for u in range(NU)]
            for g in range(4):
                for i in range(4 * g, 4 * g + 4):
                    emit_ln_tile(i)
                emit_proj_sl(wt0, QT0, g, bcol=0)

            # ---- head 0: K, V, Wout ----
            wt = load_w(wk_ext, 0)
            KT0 = [qp.tile([P, S], f16, tag=f"kt{u}", name=f"kt{u}") for u in range(NU)]
            for sl in range(4):
                emit_proj_sl(wt, KT0, sl)
            V0, wtv = emit_v_prep(0)
            emit_v_tile(V0, wtv, 0)
            emit_v_tile(V0, wtv, 1)
            wo_t0 = load_wo(0)

            # ---- head 0 attention, software-pipelined by one stage;
            #      remaining V tiles interleaved as PE filler ----
            pend = None
            vnext = 2
            for i in range(NS):
                cur = (0, i) + emit_scores(i, QT0, KT0) + (V0, wo_t0)
                for _ in range(2):
                    if vnext < NS:
                        emit_v_tile(V0, wtv, vnext)
                        vnext += 1
                if pend is not None:
                    emit_tail(*pend)
                pend = cur

            # ---- head 1 Q/K projections fill the last softmax stall ----
            wt = load_w(wq_ext, 1)
            QT1 = [qp.tile([P, S], f16, tag=f"qt{u}", name=f"qt{u}") for u in range(NU)]
            for sl in range(4):
                emit_proj_sl(wt, QT1, sl, bcol=NU)
            wt = load_w(wk_ext, 1)
            KT1 = [qp.tile([P, S], f16, tag=f"kt{u}", name=f"kt{u}") for u in range(NU)]
            for sl in range(4):
                emit_proj_sl(wt, KT1, sl)
            emit_tail(*pend)  # head-0 i=15: must precede V1 overwrite of v tags
            V1, wtv = emit_v_prep(1)
            emit_v_tile(V1, wtv, 0)
            emit_v_tile(V1, wtv, 1)
            wo_t1 = load_wo(1)

            # ---- head 1 attention ----
            pend = None
            vnext = 2
            for i in range(NS):
                cur = (1, i) + emit_scores(i, QT1, KT1) + (V1, wo_t1)
                for _ in range(2):
                    if vnext < NS:
                        emit_v_tile(V1, wtv, vnext)
                        vnext += 1
                if pend is not None:
                    emit_tail(*pend)
                pend = cur
            emit_tail(*pend)
    return nc


_NC = None


def _get_nc():
    global _NC
    if _NC is None:
        _NC = _build()
    return _NC


def _mask_np():
    r = np.arange(P)[:, None]
    j = np.arange(D)[None, :]
    m = np.empty((P, 4 * D), np.float32)
    for k in range(4):
        m[:, k * D:(k + 1) * D] = np.where(j <= k * P + r, 0.0, NEG)
    return m


def _run(inputs, trace=False):
    x = np.asarray(inputs["x"], dtype=np.float32)          # [4, 2048, 512]
    gamma = np.asarray(inputs["gamma"], dtype=np.float32).reshape(D)
    beta = np.asarray(inputs["beta"], dtype=np.float32).reshape(D)
    Wq = np.asarray(inputs["Wq"], dtype=np.float32)        # [4, 512, 1024]
    Wk = np.asarray(inputs["Wk"], dtype=np.float32)
    Wv = np.asarray(inputs["Wv"], dtype=np.float32)
    Wout = np.asarray(inputs["Wout"], dtype=np.float32)    # [4096, 512]
    mask = _mask_np()

    # fold LN gamma into projection weights; beta terms:
    #  - K bias shifts each score row by a constant -> cancels in softmax
    #  - V bias passes through softmax (rows sum to 1) -> host-side constant
    #  - Q bias added in-kernel during psum evacuation
    Wqf = Wq * gamma[None, :, None]
    Wkf = Wk * gamma[None, :, None]
    Wvf = Wv * gamma[None, :, None]
    bq_all = np.einsum("d,hdu->hu", beta, Wq)              # [4, 1024]
    bv_all = np.einsum("d,hdu->hu", beta, Wv)              # [4, 1024]
    cvec = np.zeros(D, np.float32)
    for h in range(4):
        cvec += bv_all[h] @ Wout[h * U:(h + 1) * U]

    in_maps = []
    for c in range(8):
        b, hp = c // 2, c % 2
        bq = bq_all[2 * hp:2 * hp + 2].reshape(2, NU, P).transpose(2, 0, 1).reshape(P, 2 * NU)
        in_maps.append({
            "x": np.ascontiguousarray(x[b]),
            "bq": np.ascontiguousarray(bq),
            "wq": np.ascontiguousarray(Wqf[2 * hp:2 * hp + 2].reshape(2 * D, U)).astype(np.float16),
            "wk": np.ascontiguousarray(Wkf[2 * hp:2 * hp + 2].reshape(2 * D, U)).astype(np.float16),
            "wv": np.ascontiguousarray(Wvf[2 * hp:2 * hp + 2].reshape(2 * D, U)).astype(np.float16),
            "wo": np.ascontiguousarray(Wout[2 * hp * U:(2 * hp + 2) * U]).astype(np.float16),
            "mask": mask,
        })
    res = run_bass_kernel_spmd(_get_nc(), in_maps, list(range(8)), trace=trace)
    out = np.empty((4, S, D), np.float32)
    for b in range(4):
        out[b] = res.results[2 * b]["out"] + res.results[2 * b + 1]["out"] + cvec[None, :]
    return out, res


def kernel(**inputs):
    out, _ = _run(inputs, trace=False)
    return out


# revision 37
# speedup vs baseline: 1.1685x; 1.0209x over previous
import numpy as np
import concourse.bass as bass
import concourse.tile as tile
from concourse import mybir
from concourse.bass_utils import run_bass_kernel_spmd
from concourse.masks import make_identity

P = 128
S = 2048
D = 512
U = 1024
NS = S // P      # 16 s-tiles
ND = D // P      # 4 d-blocks
NU = U // P      # 8 u-blocks
NEG = -60000.0
EPS = 1e-6


def _patched_drain_and_barrier(self, tick_clock, wait_clock):
    nc = self.nc
    probe = nc.sync.nop(nofuse=True, hint="drain_waits_probe")
    wait_clock.add_sem_waits(probe.ins, tile.ScopedClock({None: tick_clock.global_clock}))
    si = probe.ins.sync_info
    waits = list(si.on_wait) if si is not None else []
    assert self.sems is not None
    handles = {h.name: h for h in self.sems.allocated().values()}
    if len(waits) > 1:
        import bass_rust
        probe.ins.sync_info = bass_rust.SyncInfo(on_wait=waits[:1], on_update=[])
        for w in waits[1:]:
            h = handles.get(w.ant_name)
            assert h is not None, (w.ant_name, list(handles))
            nc.sync.wait_ge(h, w.wait_value)
    nc.sync.drain()
    nc.all_engine_barrier()
    popped = nc._tile_sem_poison_stack.pop()
    assert popped is self._sem_poison
    nc.clear_and_free_semaphores(list(self.sems.allocated().values()))
    nc.all_engine_barrier()


tile.TileContext._drain_and_barrier = _patched_drain_and_barrier

# The walrus backend in this toolchain rejects instructions carrying more
# than one semaphore wait ("Too many sync wait commands"). Split excess
# waits onto single-wait NoOp carriers on the same engine, which execute
# in order ahead of the real instruction.
_MAXW = 1
_orig_lower_ordered = tile.TileContext._lower_ordered_insts


def _patched_lower_ordered(self, ordered):
    nc = self.nc
    for insts in ordered.values():
        out = []
        for inst in insts:
            si = getattr(inst, "sync_info", None)
            eng = getattr(inst, "engine", None)
            if (si is not None and si.on_wait and len(si.on_wait) > _MAXW
                    and eng is not None
                    and not type(inst).__name__.startswith("BassTile")):
                waits = list(si.on_wait)
                for w in waits[:-_MAXW]:
                    out.append(mybir.InstNoOp(
                        name=nc.get_next_instruction_name(),
                        engine=eng,
                        ins=[],
                        outs=[],
                        bass_nofuse=True,
                        sync_info=mybir.SyncInfo(on_wait=[w], on_update=[]),
                    ))
                inst.sync_info = mybir.SyncInfo(
                    on_wait=waits[-_MAXW:], on_update=list(si.on_update))
            out.append(inst)
        insts[:] = out
    return _orig_lower_ordered(self, ordered)


tile.TileContext._lower_ordered_insts = _patched_lower_ordered

f32 = mybir.dt.float32
f16 = mybir.dt.float16


def _build():
    nc = bass.Bass()
    x_ext = nc.declare_dram_parameter("x", [S, D], f32, isOutput=False)
    bq_ext = nc.declare_dram_parameter("bq", [P, 2 * NU], f32, isOutput=False)
    wq_ext = nc.declare_dram_parameter("wq", [2 * D, U], f16, isOutput=False)
    wk_ext = nc.declare_dram_parameter("wk", [2 * D, U], f16, isOutput=False)
    wv_ext = nc.declare_dram_parameter("wv", [2 * D, U], f16, isOutput=False)
    wo_ext = nc.declare_dram_parameter("wo", [2 * U, D], f16, isOutput=False)
    out_ext = nc.declare_dram_parameter("out", [S, D], f32, isOutput=True)

    with tile.TileContext(nc) as tc:
        with tc.tile_pool(name="const", bufs=1) as cp, \
             tc.tile_pool(name="xnt", bufs=1) as xp, \
             tc.tile_pool(name="wp", bufs=1) as wp, \
             tc.tile_pool(name="wop", bufs=1) as wop, \
             tc.tile_pool(name="qkv", bufs=1) as qp, \
             tc.tile_pool(name="ln", bufs=2) as lp, \
             tc.tile_pool(name="xd", bufs=6) as xdp, \
             tc.tile_pool(name="att", bufs=2) as ap_, \
             tc.tile_pool(name="st", bufs=2) as sp, \
             tc.tile_pool(name="oacc", bufs=1) as op, \
             tc.tile_pool(name="outp", bufs=2) as up, \
             tc.tile_pool(name="mm", bufs=2, space="PSUM") as mmp, \
             tc.tile_pool(name="sc", bufs=2, space="PSUM") as scp, \
             tc.tile_pool(name="pv", bufs=1, space="PSUM") as pvp, \
             tc.tile_pool(name="tr", bufs=2, space="PSUM") as trp:

            ident = cp.tile([P, P], f16, tag="ident")
            make_identity(nc, ident[:])
            bqt = cp.tile([P, 2 * NU], f32, tag="bqt")
            nc.sync.dma_start(out=bqt[:], in_=bq_ext[:, :])
            eps = cp.tile([P, 1], f32, tag="eps")
            nc.vector.memset(eps[:], EPS)
            mask = cp.tile([P, 4 * D], f16, tag="mask")

            xnT = [xp.tile([P, S], f16, tag=f"xnt{j}", name=f"xnt{j}") for j in range(ND)]
            oacc = [op.tile([P, D], f16, tag=f"oacc{i}", name=f"oacc{i}") for i in range(NS)]

            dmaq = [nc.sync, nc.scalar, nc.gpsimd]
            xq = {}
            _oth = [t for t in range(NS) if t not in (5, 8, 11, 14)]
            for _k, _t in enumerate(_oth):
                xq[_t] = _k % 2
            for _t in (5, 8, 11, 14):
                xq[_t] = 2

            def emit_ln_tile(i):
                xt = xdp.tile([P, D], f32, tag="x", name="xt")
                dmaq[xq[i]].dma_start(out=xt[:], in_=x_ext[i * P:(i + 1) * P, :])
                stats = lp.tile([P, 6], f32, tag="bs", name="bs")
                nc.vector.bn_stats(out=stats[:], in_=xt[:])
                mv = lp.tile([P, 2], f32, tag="mv", name="mv")
                nc.vector.bn_aggr(out=mv[:], in_=stats[:])
                sd = lp.tile([P, 1], f32, tag="sd", name="sd")
                nc.scalar.activation(out=sd[:], in_=mv[:, 1:2],
                                     func=mybir.ActivationFunctionType.Sqrt,
                                     bias=eps[:], scale=1.0, alpha=0.0)
                nc.vector.reciprocal(out=sd[:], in_=sd[:])
                xh = lp.tile([P, D], f16, tag="xh", name="xh")
                nc.vector.tensor_scalar(out=xh[:], in0=xt[:],
                                        scalar1=mv[:, 0:1], scalar2=sd[:],
                                        op0=mybir.AluOpType.subtract,
                                        op1=mybir.AluOpType.mult)
                for j in range(ND):
                    tp = trp.tile([P, P], f16, tag="tr", name="tp")
                    nc.tensor.transpose(tp[:], xh[:, j * P:(j + 1) * P], ident[:])
                    nc.any.tensor_copy(out=xnT[j][:, i * P:(i + 1) * P], in_=tp[:])

            def load_w(w_ext_, h):
                wt = [wp.tile([P, U], f16, tag=f"w{j}", name=f"w{j}") for j in range(ND)]
                for j in range(ND):
                    nc.gpsimd.dma_start(
                        out=wt[j][:],
                        in_=w_ext_[h * D + j * P: h * D + (j + 1) * P, :])
                return wt

            def emit_proj_sl(wt, dst, sl, bcol=None):
                for u in range(NU):
                    mm = mmp.tile([P, 512], f32, tag="mm", name="mm")
                    for j in range(ND):
                        nc.tensor.matmul(mm[:],
                                         wt[j][:, u * P:(u + 1) * P],
                                         xnT[j][:, sl * 512:(sl + 1) * 512],
                                         start=(j == 0), stop=(j == ND - 1))
                    if bcol is None:
                        nc.any.tensor_copy(out=dst[u][:, sl * 512:(sl + 1) * 512], in_=mm[:])
                    else:
                        nc.any.tensor_scalar_add(out=dst[u][:, sl * 512:(sl + 1) * 512],
                                                 in0=mm[:],
                                                 scalar1=bqt[:, bcol + u:bcol + u + 1])

            def emit_v_prep(h):
                V = [qp.tile([P, U], f16, tag=f"v{t}", name=f"v{t}") for t in range(NS)]
                wt = [wp.tile([P, U], f16, tag=f"w{j}", name=f"w{j}") for j in range(ND)]
                for j in range(ND):
                    nc.gpsimd.dma_start(
                        out=wt[j][:],
                        in_=wv_ext[h * D + j * P: h * D + (j + 1) * P, :])
                return V, wt

            def emit_v_tile(V, wt, t):
                for us in range(2):
                    mm = mmp.tile([P, 512], f32, tag="mm", name="mm")
                    for j in range(ND):
                        nc.tensor.matmul(mm[:],
                                         xnT[j][:, t * P:(t + 1) * P],
                                         wt[j][:, us * 512:(us + 1) * 512],
                                         start=(j == 0), stop=(j == ND - 1))
                    nc.any.tensor_copy(out=V[t][:, us * 512:(us + 1) * 512], in_=mm[:])

            def load_wo(h):
                wo_t = [wop.tile([P, D], f16, tag=f"wo{ub}", name=f"wo{ub}") for ub in range(NU)]
                for ub in range(NU):
                    nc.gpsimd.dma_start(
                        out=wo_t[ub][:],
                        in_=wo_ext[h * U + ub * P: h * U + (ub + 1) * P, :])
                return wo_t

            def emit_scores(i, QT, KT):
                nch = i // 4 + 1
                Pt = ap_.tile([P, S], f16, tag="P", name="Pt")
                mneg = sp.tile([P, 4], f32, tag="mneg", name="mneg")
                rsum = sp.tile([P, 4], f32, tag="rsum", name="rsum")
                for c in range(nch):
                    w = (i % 4 + 1) * P if c == i // 4 else 512
                    sc = scp.tile([P, 512], f32, tag="sc", name="sc")
                    for u in range(NU):
                        nc.tensor.matmul(sc[:, 0:w],
                                         QT[u][:, i * P:(i + 1) * P],
                                         KT[u][:, c * 512:c * 512 + w],
                                         start=(u == 0), stop=(u == NU - 1))
                    if c == i // 4:
                        m = i % 4
                        nc.vector.tensor_add(out=sc[:, 0:w], in0=sc[:, 0:w],
                                             in1=mask[:, m * 512:m * 512 + w])
                    nc.vector.reduce_max(out=mneg[:, c:c + 1], in_=sc[:, 0:w],
                                         axis=mybir.AxisListType.X, negate=True)
                    nc.scalar.activation(out=Pt[:, c * 512:c * 512 + w], in_=sc[:, 0:w],
                                         func=mybir.ActivationFunctionType.Exp,
                                         bias=mneg[:, c:c + 1], scale=1.0,
                                         accum_out=rsum[:, c:c + 1])
                return Pt, mneg, rsum

            def emit_tail(h, i, Pt, mneg, rsum, V, wo_t, final=False):
                nch = i // 4 + 1
                # global softmax rescale: beta_c = exp(m_c - m_g) / Z
                mpos = sp.tile([P, 4], f32, tag="mpos", name="mpos")
                nc.vector.tensor_scalar_mul(out=mpos[:, 0:nch], in0=mneg[:, 0:nch],
                                            scalar1=-1.0)
                mgn = sp.tile([P, 1], f32, tag="mgn", name="mgn")
                nc.vector.reduce_max(out=mgn[:], in_=mpos[:, 0:nch],
                                     axis=mybir.AxisListType.X, negate=True)
                alph = sp.tile([P, 4], f32, tag="alph", name="alph")
                nc.scalar.activation(out=alph[:, 0:nch], in_=mneg[:, 0:nch],
                                     func=mybir.ActivationFunctionType.Exp,
                                     bias=mgn[:], scale=-1.0)
                pr = sp.tile([P, 4], f32, tag="pr", name="pr")
                nc.vector.tensor_mul(out=pr[:, 0:nch], in0=rsum[:, 0:nch],
                                     in1=alph[:, 0:nch])
                tot = sp.tile([P, 1], f32, tag="tot", name="tot")
                nc.vector.reduce_sum(out=tot[:], in_=pr[:, 0:nch],
                                     axis=mybir.AxisListType.X)
                nc.vector.reciprocal(out=tot[:], in_=tot[:])
                bt = sp.tile([P, 4], f32, tag="bt", name="bt")
                nc.vector.tensor_scalar_mul(out=bt[:, 0:nch], in0=alph[:, 0:nch],
                                            scalar1=tot[:])
                for c in range(nch):
                    w = (i % 4 + 1) * P if c == i // 4 else 512
                    nc.vector.tensor_scalar_mul(out=Pt[:, c * 512:c * 512 + w],
                                                in0=Pt[:, c * 512:c * 512 + w],
                                                scalar1=bt[:, c:c + 1])
                # transpose probs blocks 0..i
                pt = ap_.tile([P, S], f16, tag="pt", name="pt")
                for tb in range(i + 1):
                    tp = trp.tile([P, P], f16, tag="tr", name="tp")
                    nc.tensor.transpose(tp[:], Pt[:, tb * P:(tb + 1) * P], ident[:])
                    nc.any.tensor_copy(out=pt[:, tb * P:(tb + 1) * P], in_=tp[:])
                # probs @ V
                ht = ap_.tile([P, U], f16, tag="ht", name="ht")
                for us in range(2):
                    pv = pvp.tile([P, 512], f32, tag=f"pv{us}", name="pv")
                    for tb in range(i + 1):
                        nc.tensor.matmul(pv[:],
                                         pt[:, tb * P:(tb + 1) * P],
                                         V[tb][:, us * 512:(us + 1) * 512],
                                         start=(tb == 0), stop=(tb == i))
                    nc.any.tensor_copy(out=ht[:, us * 512:(us + 1) * 512], in_=pv[:])
                # transpose head-out blocks
                htt = ap_.tile([P, U], f16, tag="htt", name="htt")
                for ub in range(NU):
                    tp = trp.tile([P, P], f16, tag="tr", name="tp")
                    nc.tensor.transpose(tp[:], ht[:, ub * P:(ub + 1) * P], ident[:])
                    nc.any.tensor_copy(out=htt[:, ub * P:(ub + 1) * P], in_=tp[:])
                # output projection
                om = mmp.tile([P, 512], f32, tag="mm", name="om")
                for ub in range(NU):
                    nc.tensor.matmul(om[:],
                                     htt[:, ub * P:(ub + 1) * P],
                                     wo_t[ub][:],
                                     start=(ub == 0), stop=(ub == NU - 1))
                if h == 0:
                    nc.any.tensor_copy(out=oacc[i][:], in_=om[:])
                else:
                    of = up.tile([P, D], f32, tag="of", name="of")
                    nc.vector.tensor_add(out=of[:], in0=om[:], in1=oacc[i][:])
                    if final:
                        nc.sync.dma_start(out=out_ext[i * P:i * P + 64, :],
                                          in_=of[0:64, :])
                        nc.scalar.dma_start(out=out_ext[i * P + 64:(i + 1) * P, :],
                                            in_=of[64:128, :])
                    else:
                        nc.sync.dma_start(out=out_ext[i * P:(i + 1) * P, :], in_=of[:])

            # ---- LayerNorm interleaved with head-0 Q projection ----
            wt0 = load_w(wq_ext, 0)
            nc.gpsimd.memset(mask[:], 0.0)
            for m in range(4):
                # keep 0 where j <= m*128 + r, else NEG
                nc.gpsimd.affine_select(
                    out=mask[:, m * 512:(m + 1) * 512],
                    in_=mask[:, m * 512:(m + 1) * 512],
                    compare_op=mybir.AluOpType.is_ge,
                    fill=NEG,
                    base=m * P,
                    pattern=[[-1, 512]],
                    channel_multiplier=1,
                )
            QT0 = [qp.tile([P, S], f16, tag=f"qt{u}", name=f"qt{u}") for u in range(NU)]
            for g in range(4):
                for i in range(4 * g, 4 * g + 4):
                    emit_ln_tile(i)
                emit_proj_sl(wt0, QT0, g, bcol=0)

            # ---- head 0: K, V, Wout ----
            wt = load_w(wk_ext, 0)
            KT0 = [qp.tile([P, S], f16, tag=f"kt{u}", name=f"kt{u}") for u in range(NU)]
            for sl in range(4):
                emit_proj_sl(wt, KT0, sl)
            V0, wtv = emit_v_prep(0)
            emit_v_tile(V0, wtv, 0)
            emit_v_tile(V0, wtv, 1)
            wo_t0 = load_wo(0)

            # ---- head 0 attention, software-pipelined by one stage;
            #      remaining V tiles interleaved as PE filler ----
            pend = None
            vnext = 2
            for i in range(NS):
                cur = (0, i) + emit_scores(i, QT0, KT0) + (V0, wo_t0)
                for _ in range(2):
                    if vnext < NS:
                        emit_v_tile(V0, wtv, vnext)
                        vnext += 1
                if pend is not None:
                    emit_tail(*pend)
                pend = cur

            # ---- head 1 Q/K projections fill the last softmax stall ----
            wt = load_w(wq_ext, 1)
            QT1 = [qp.tile([P, S], f16, tag=f"qt{u}", name=f"qt{u}") for u in range(NU)]
            for sl in range(4):
                emit_proj_sl(wt, QT1, sl, bcol=NU)
            wt = load_w(wk_ext, 1)
            KT1 = [qp.tile([P, S], f16, tag=f"kt{u}", name=f"kt{u}") for u in range(NU)]
            for sl in range(4):
                emit_proj_sl(wt, KT1, sl)
            emit_tail(*pend)  # head-0 i=15: must precede V1 overwrite of v tags
            V1, wtv = emit_v_prep(1)
            emit_v_tile(V1, wtv, 0)
            emit_v_tile(V1, wtv, 1)
            wo_t1 = load_wo(1)

            # ---- head 1 attention ----
            pend = None
            vnext = 2
            for i in range(NS):
                cur = (1, i) + emit_scores(i, QT1, KT1) + (V1, wo_t1)
                for _ in range(2):
                    if vnext < NS:
                        emit_v_tile(V1, wtv, vnext)
                        vnext += 1
                if pend is not None:
                    emit_tail(*pend)
                pend = cur
            emit_tail(*pend, final=True)
    return nc


_NC = None


def _get_nc():
    global _NC
    if _NC is None:
        _NC = _build()
    return _NC


def _run(inputs, trace=False):
    x = np.asarray(inputs["x"], dtype=np.float32)          # [4, 2048, 512]
    gamma = np.asarray(inputs["gamma"], dtype=np.float32).reshape(D)
    beta = np.asarray(inputs["beta"], dtype=np.float32).reshape(D)
    Wq = np.asarray(inputs["Wq"], dtype=np.float32)        # [4, 512, 1024]
    Wk = np.asarray(inputs["Wk"], dtype=np.float32)
    Wv = np.asarray(inputs["Wv"], dtype=np.float32)
    Wout = np.asarray(inputs["Wout"], dtype=np.float32)    # [4096, 512]

    # fold LN gamma into projection weights; beta terms:
    #  - K bias shifts each score row by a constant -> cancels in softmax
    #  - V bias passes through softmax (rows sum to 1) -> host-side constant
    #  - Q bias added in-kernel during psum evacuation
    Wqf = Wq * gamma[None, :, None]
    Wkf = Wk * gamma[None, :, None]
    Wvf = Wv * gamma[None, :, None]
    bq_all = np.einsum("d,hdu->hu", beta, Wq)              # [4, 1024]
    bv_all = np.einsum("d,hdu->hu", beta, Wv)              # [4, 1024]
    cvec = np.zeros(D, np.float32)
    for h in range(4):
        cvec += bv_all[h] @ Wout[h * U:(h + 1) * U]

    in_maps = []
    for c in range(8):
        b, hp = c // 2, c % 2
        bq = bq_all[2 * hp:2 * hp + 2].reshape(2, NU, P).transpose(2, 0, 1).reshape(P, 2 * NU)
        in_maps.append({
            "x": np.ascontiguousarray(x[b]),
            "bq": np.ascontiguousarray(bq),
            "wq": np.ascontiguousarray(Wqf[2 * hp:2 * hp + 2].reshape(2 * D, U)).astype(np.float16),
            "wk": np.ascontiguousarray(Wkf[2 * hp:2 * hp + 2].reshape(2 * D, U)).astype(np.float16),
            "wv": np.ascontiguousarray(Wvf[2 * hp:2 * hp + 2].reshape(2 * D, U)).astype(np.float16),
            "wo": np.ascontiguousarray(Wout[2 * hp * U:(2 * hp + 2) * U]).astype(np.float16),
        })
    res = run_bass_kernel_spmd(_get_nc(), in_maps, list(range(8)), trace=trace)
    out = np.empty((4, S, D), np.float32)
    for b in range(4):
        out[b] = res.results[2 * b]["out"] + res.results[2 * b + 1]["out"] + cvec[None, :]
    return out, res


def kernel(**inputs):
    out, _ = _run(inputs, trace=False)
    return out
